# revision 69
# baseline (speedup 1.0000x reference)
"""Trainium2 Bass kernel for nn_ActionPredictionNet (GNN message passing).

Data-parallel over batch*particles: 8 NeuronCores, each handling 256
independent fully-connected 10-node particle graphs (2560 nodes, 23040
edges). The fully-connected structure lets us restructure the math:

  - Edge-MLP layer 1 collapses: e_in = [n[s], n[r]] so layer-1 pre-act is
    u[s] + v[r] with u = W_top^T n, v = W_bot^T n computed per NODE
    (2560 cols) instead of per EDGE (23040 cols), then a broadcast-add.
  - Edges are only consumed via the mean over incoming messages, so edge
    layer 3 folds into the aggregation: accumulate (sum_s h2_s) @ (w_e3/9)
    in PSUM. The aggregation matmuls run in fp8 DoubleRow mode (two sender
    slots per pass), halving their PE time; h2 is stored fp8 with a 4x
    scale folded into w_e2/b_e2 and 64x into wn1b, compensated by a 1/256
    scale on the n1 eviction.
  - Diagonal (s == r) pairs are never computed: per receiver the sender
    range splits into two dense pieces.

Layouts (per core, feat-major: features on SBUF partitions):
  - node tensors [128, 2560], column = a*256 + p  (a: node-in-graph 0..9,
    p: graph 0..255)  -> broadcast APs get innermost unit stride.
  - edge tensors [128, 23040], column = r*2304 + s'*256 + p (s' skips r).

Schedule notes (from perfetto traces of the previous version):
  - input DMA is issued from three engines in parallel (scalar / sync /
    gpsimd) so the first enc matmul can start ~5us in instead of ~11us.
  - PE HAM warm-up fillers accumulate into a dedicated PSUM bank (no
    eviction sink needed); in the e2 stream they reuse the currently
    loaded stationary weights so they cost no LDWEIGHTS.
  - PSUM evictions are the bottleneck (~1.1-1.3 ns/col on ACT/DVE, PSUM
    read port is 1 elem/cycle); they are batched at FD=1280 and routed
    across ACT/DVE by tunable tables; h1 relus run on DVE (fp16 4x mode),
    optionally a few on GPSIMD.
"""

import numpy as np

B, P, A = 32, 64, 10
S_DIM, H_DIM, MID = 64, 64, 128
ACT = 8
N_CORES = 8
NP_CORE = B * P // N_CORES          # 256 particle-graphs per core
NODES = NP_CORE * A                 # 2560 nodes per core
QB = (A - 1) * NP_CORE              # 2304 edge columns per receiver block
ECOLS = A * QB                      # 23040 (r, s', p) edge columns per core

GW = 1024                           # node-layer eviction group width
# e2 groups alternate between a 3-bank and a 2-bank PSUM tile: 9 pairs of
# (1536, 1024) = exactly 23040 edge cols, 18 evictions instead of 23
EGW = [1536, 1024] * 9
N_EG = len(EGW)
EG0 = [sum(EGW[:g]) for g in range(N_EG)]

# fp8 scaling for the aggregation path
S_H2 = 4.0                          # h2 stored as 4*h2 (folded into w_e2/b_e2)
S_WB = 64.0                         # wn1b stored as 64*wn1b
S_N1 = 1.0 / (S_H2 * S_WB)          # eviction scale on the n1 pre-act
S_W1 = 16.0                         # w_in1 stored fp8 as 16*w_in1

_PROG = None        # cached compiled program: (nc, meta)
LAST_EXEC_NS = None  # filled when KERNEL_TRACE=1


# ------------------------------------------------------------ tuning tables
# eviction engine per group: enc(2), u(2), v(2), h2(18); relu engine per r
EV_ENC = ["act", "vec", "act"]
EV_U = ["vec", "act", "vec"]
EV_V = ["act", "vec", "vec"]
EV_H2 = ["act"] * 15 + ["act", "vec", "act"]
RELU_ENG = ["act", "vec", "vec", "vec", "vec", "vec",
            "vec", "vec", "vec", "vec"]  # per receiver block
EV_N2 = "vec"
EV_L1 = ["vec", "vec", "vec", "vec", "vec"]   # per 512-col slab step
EV_OUT = "vec"


# ---------------------------------------------------------------- host utils

def _expected_edges():
    a = np.arange(A)
    s, r = np.meshgrid(a, a, indexing="ij")
    m = s != r
    s, r = s[m], r[m]
    offs = (np.arange(B * P) * A)[:, None]
    return (offs + s[None, :]).reshape(-1).astype(np.int64), \
           (offs + r[None, :]).reshape(-1).astype(np.int64)


def _to_ap_major(x_core):
    """[2560, D] in (p, a) node order -> [D, 2560] feat-major, (a, p) cols."""
    return np.ascontiguousarray(
        x_core.reshape(NP_CORE, A, -1).transpose(1, 0, 2).reshape(NODES, -1).T
    )


def _from_ap_major(out_core):
    """[ACT, 2560] feat-major (a, p) cols -> [2560, ACT] in (p, a) order."""
    return out_core.T.reshape(A, NP_CORE, ACT).transpose(1, 0, 2).reshape(NODES, ACT)


def _fallback_numpy(theta, s, i, senders, receivers,
                    w_in1, b_in1, w_in2, b_in2,
                    w_e1, b_e1, w_e2, b_e2, w_e3, b_e3,
                    w_n1, b_n1, w_n2, b_n2, w_n3, b_n3,
                    w_l1, b_l1, w_l2, b_l2):
    """fp32 numpy replica of the reference; used only if inputs deviate from
    the documented structure (non-fully-connected edges or non-constant i)."""
    N = B * P * A
    relu = lambda x: np.maximum(x, 0.0)
    x = np.concatenate([theta.reshape(N, H_DIM), s.reshape(N, S_DIM),
                        i.reshape(N, 1)], axis=-1).astype(np.float32)
    n = relu(x @ w_in1 + b_in1) @ w_in2 + b_in2
    e_in = np.concatenate([n[senders], n[receivers]], axis=-1)
    e = relu(e_in @ w_e1 + b_e1)
    e = relu(e @ w_e2 + b_e2)
    e = e @ w_e3 + b_e3
    agg = np.zeros((N, e.shape[1]), np.float32)
    np.add.at(agg, receivers, e)
    agg /= (A - 1)
    h = np.concatenate([n, agg], axis=-1)
    h = relu(h @ w_n1 + b_n1)
    h = relu(h @ w_n2 + b_n2)
    h = h @ w_n3 + b_n3
    out = relu(h @ w_l1 + b_l1) @ w_l2 + b_l2
    return out.reshape(B, P, A, ACT).astype(np.float32)


# ------------------------------------------------------------- device program

# fp16 weight-pack slot indices (linear-linear layer pairs folded on host:
# w_in2 into we1t/we1b/wn1a, w_n3 into w_l1; w_e3/9 lives in the fp8 pack)
W1T, WE1T, WE1B, WE2S, WN1AS, WN2, WL1, WL2 = range(8)
NSLOTS = 8
# bias-pack column indices
B1, BU, BV, BE2S, BN1, BN2, BL1, BL2 = range(8)


def _build_program():
    import concourse.bass as bass
    import concourse.mybir as mybir
    import concourse.tile as tile
    from concourse import bacc

    f16 = mybir.dt.float16
    f32 = mybir.dt.float32
    f8 = mybir.dt.float8e4
    Af = mybir.ActivationFunctionType
    Op = mybir.AluOpType
    DR = mybir.MatmulPerfMode.DoubleRow

    nc = bacc.Bacc("TRN2", target_bir_lowering=False, debug=False)
    x_dram = nc.dram_tensor("x_fm", [128, NODES], f16, kind="ExternalInput").ap()
    w_dram = nc.dram_tensor("w_pack", [128, NSLOTS * 128], f16,
                            kind="ExternalInput").ap()
    wdr_dram = nc.dram_tensor("wdr_pack", [128, 384], f8,
                              kind="ExternalInput").ap()
    b_dram = nc.dram_tensor("b_pack", [128, 8], f32, kind="ExternalInput").ap()
    out_dram = nc.dram_tensor("out", [ACT, NODES], f32, kind="ExternalOutput").ap()

    with tile.TileContext(nc) as tc:
        with (
            tc.tile_pool(name="consts", bufs=1) as consts,
            tc.tile_pool(name="bigs", bufs=1) as bigs,
            tc.tile_pool(name="psAa", bufs=1, space="PSUM") as psAa,
            tc.tile_pool(name="psAb", bufs=1, space="PSUM") as psAb,
            tc.tile_pool(name="psB", bufs=2, space="PSUM") as psB,
            tc.tile_pool(name="psF", bufs=1, space="PSUM") as psF,
        ):
            wt = consts.tile([128, NSLOTS * 128], f16, tag="wt")
            wdr = consts.tile([128, 384], f8, tag="wdr")
            bt = consts.tile([128, 8], f32, tag="bt")
            x_fm = bigs.tile([128, NODES], f16, tag="x_fm")
            dummy = consts.tile([128, 256], f16, tag="dummy")
            dsink = consts.tile([128, 8], f32, tag="dsink")

            # ---- input DMA, spread across three issuing engines so the
            # transfers overlap; first enc group only needs x[:, :1280].
            nc.gpsimd.memset(dummy[:], 0.0)
            nc.gpsimd.dma_start(out=x_fm[:, 1920:2560], in_=x_dram[:, 1920:2560])
            nc.scalar.dma_start(out=x_fm[:, 0:1280], in_=x_dram[:, 0:1280])
            nc.scalar.dma_start(out=bt[:], in_=b_dram)
            nc.sync.dma_start(out=wt[:, :3 * 128], in_=w_dram[:, :3 * 128])
            nc.sync.dma_start(out=x_fm[:, 1280:1920], in_=x_dram[:, 1280:1920])
            nc.sync.dma_start(out=wt[:, 3 * 128:], in_=w_dram[:, 3 * 128:])
            nc.sync.dma_start(out=wdr[:], in_=wdr_dram)

            # touch the Relu table set early so ACT_TABLE_LOAD hides in the
            # DMA-wait head instead of stalling the first real eviction
            nc.scalar.activation(dsink[:, 0:1], dummy[:, 0:2].bitcast(f32),
                                 Af.Relu)

            W = lambda k: wt[:, k * 128:(k + 1) * 128]
            bias = lambda k: bt[:, k:k + 1]
            wdr_pair = wdr[:, 0:256].rearrange("f (j m) -> f j m", j=2)
            wdr_one = wdr[:, 256:384]

            # ---- HAM warm-up fillers: accumulate into a dedicated PSUM
            # bank, no eviction needed. `w_ap`/`src` choose the stationary /
            # moving operands: reusing the neighbouring real matmuls'
            # stationary makes a filler cost zero LDWEIGHTS; reading freshly
            # produced data pins the filler to that point of the pipeline.
            fps = psF.tile([128, 512], f32, tag="psF")

            def pe_filler(n=1, w_ap=None, src=None):
                mv = src if src is not None else dummy[:, :256]
                st = w_ap if w_ap is not None else dummy[:, :128]
                fd = mv.shape[-1]
                for _ in range(n):
                    nc.tensor.matmul(fps[:, :fd], st, mv,
                                     start=True, stop=True,
                                     skip_group_check=True)

            pe_filler(22)  # warm-up bridging the whole input-DMA wait:
                           # PE must be at 2.4 GHz when x lands (~10us)

            t_enc = bigs.tile([128, NODES], f16, tag="t_enc")
            u_t = bigs.tile([128, NODES], f16, tag="u_t")
            v_t = bigs.tile([128, NODES], f16, tag="v_t")
            h1_t = bigs.tile([128, ECOLS], f16, tag="h1_t")
            h2_t = bigs.tile([128, ECOLS], f8, tag="h2_t")
            t_n1 = bigs.tile([128, NODES], f16, tag="t_n1")
            t_n2 = bigs.tile([128, NODES], f16, tag="t_n2")
            t_l1 = bigs.tile([128, NODES], f16, tag="t_l1")
            out_sb = bigs.tile([ACT, NODES], f32, tag="out_sb")

            def evict(eng, dst, src, bias_ap, relu, scale=None):
                if eng == "act":
                    if scale is not None:
                        nc.scalar.activation(dst, src, Af.Relu if relu else
                                             Af.Identity, bias=bias_ap,
                                             scale=scale)
                    elif relu:
                        nc.scalar.activation(dst, src, Af.Relu, bias=bias_ap)
                    elif bias_ap is not None:
                        nc.scalar.activation(dst, src, Af.Identity,
                                             bias=bias_ap)
                    else:
                        nc.scalar.copy(dst, src)
                else:
                    assert scale is None
                    if relu:
                        nc.vector.tensor_scalar(dst, src, bias_ap, 0.0,
                                                Op.add, Op.max)
                    elif bias_ap is not None:
                        nc.vector.tensor_scalar_add(dst, src, bias_ap)
                    else:
                        nc.vector.tensor_copy(dst, src)

            def egroup_tile(gi):
                if gi % 2 == 0:
                    pst = psAa.tile([128, 1536], f32, tag="psAa")
                else:
                    pst = psAb.tile([128, 1024], f32, tag="psAb")
                return pst

            def node_layer(w_ap, src, dst, bias_idx, relu, engines,
                           scale=None):
                """2560-col dense layer as FD<=1024 PSUM groups."""
                for gi, g0 in enumerate(range(0, NODES, GW)):
                    gw = min(GW, NODES - g0)
                    ps = egroup_tile(gi)
                    for o in range(0, gw, 512):
                        nw = min(512, gw - o)
                        nc.tensor.matmul(ps[:, o:o + nw], w_ap,
                                         src[:, g0 + o:g0 + o + nw],
                                         start=True, stop=True)
                    evict(engines[gi], dst[:, g0:g0 + gw], ps[:, :gw],
                          bias(bias_idx), relu, scale=scale)

            # ---- node encoder + edge layer-1 node halves (w_in2 folded in);
            # u before v: u's eviction tail overlaps v's matmuls, and the
            # adds only need v's first group to start.
            node_layer(W(W1T), x_fm, t_enc, B1, True, EV_ENC)
            pe_filler(1, src=t_enc[:, 0:256])
            node_layer(W(WE1T), t_enc, u_t, BU, False, EV_U)
            pe_filler(1, src=u_t[:, 0:256])
            node_layer(W(WE1B), t_enc, v_t, BV, False, EV_V)
            pe_filler(1, src=v_t[:, 0:256])

            # ---- h1 = relu(u[s] + v[r]) over (r, s', p) columns, where the
            # 9 sender slots s' skip s == r (no diagonal is ever computed).
            v3 = v_t[:].rearrange("f (r p) -> f r p", p=NP_CORE)

            def tt_r(r):
                w0 = r * QB
                vb1 = v3[:, r:r + 1, :]
                ranges = [(0, r, w0), (r + 1, A, w0 + r * NP_CORE)]
                if r < 2:
                    # split the long range at the u-eviction-group boundary
                    # (s-block 8 = u col 2048) so the first piece only waits
                    # on u groups 0-1 and the adds start ~1us earlier
                    lo, hi, d0 = ranges.pop()
                    ranges += [(lo, 8, d0), (8, hi, d0 + (8 - lo) * NP_CORE)]
                first = True
                for lo, hi, d0 in ranges:
                    k = hi - lo
                    if k == 0:
                        continue
                    o = h1_t[:, d0:d0 + k * NP_CORE] \
                        .rearrange("f (s p) -> f s p", p=NP_CORE)
                    us = u_t[:, lo * NP_CORE:hi * NP_CORE] \
                        .rearrange("f (s p) -> f s p", p=NP_CORE)
                    nc.vector.tensor_add(o, us,
                                         vb1.broadcast_to([128, k, NP_CORE]))
                    if r < 2 and first:
                        pe_filler(1, w_ap=W(WE2S),
                                  src=h1_t[:, d0:d0 + 128])
                    first = False

                def relu_part(flat):
                    eng = RELU_ENG[r]
                    if eng == "act":
                        nc.scalar.activation(flat, flat, Af.Relu)
                    else:
                        nc.vector.tensor_scalar_max(flat, flat, 0.0)
                if r < 2:
                    # split so the first e2 groups unblock sooner
                    relu_part(h1_t[:, w0:w0 + GW])
                    relu_part(h1_t[:, w0 + GW:w0 + QB])
                else:
                    relu_part(h1_t[:, w0:w0 + QB])

            pe_filler(1, src=u_t[:, 2304:2560])
            pe_filler(1, src=v_t[:, 2304:2560])
            for r in range(A):
                tt_r(r)
                if r < 3:
                    pe_filler(2, src=h1_t[:, r * QB:r * QB + 256])

            # ---- main stream: h2 = relu(w_e2s^T h1 + b_e2s) stored fp8;
            # after the groups covering r-blocks {2c, 2c+1}, the fused
            # agg+n1 chunk runs (fp8 DoubleRow over sender-slot pairs), and
            # per completed t_n1 slab the rest of the network.
            h2v = h2_t[:].rearrange("f (r s p) -> f s r p", s=A - 1,
                                    p=NP_CORE)

            def agg_chunk(c):
                ps = psB.tile([128, 512], f32, tag="psB")
                nc.tensor.matmul(ps[:], W(WN1AS),
                                 t_enc[:, c * 512:(c + 1) * 512],
                                 start=True, stop=False)
                for ri, r in enumerate((2 * c, 2 * c + 1)):
                    po = ri * NP_CORE
                    for a2 in range(4):
                        nc.tensor.matmul(
                            ps[:, po:po + NP_CORE], wdr_pair,
                            h2v[:, 2 * a2:2 * a2 + 2, r:r + 1, :],
                            start=False, stop=False, perf_mode=DR)
                    nc.tensor.matmul(ps[:, po:po + NP_CORE], wdr_one,
                                     h2v[:, 8:9, r:r + 1, :],
                                     start=False, stop=(ri == 1))
                evict("act", t_n1[:, c * 512:(c + 1) * 512], ps[:],
                      bias(BN1), True, scale=S_N1)

            def slab(s0, sw, step):
                """node-MLP tail + decoder for t_n1 cols [s0, s0+sw).
                Stage-major over the slab so each weight loads once."""
                steps = list(range(s0, s0 + sw, step))
                for si, c0 in enumerate(steps):
                    ps = psB.tile([128, 512], f32, tag="psB")
                    nc.tensor.matmul(ps[:, :step], W(WN2),
                                     t_n1[:, c0:c0 + step],
                                     start=True, stop=True)
                    evict(EV_N2, t_n2[:, c0:c0 + step], ps[:, :step],
                          bias(BN2), True)
                for si, c0 in enumerate(steps):
                    ps = psB.tile([128, 512], f32, tag="psB")
                    nc.tensor.matmul(ps[:, :step], W(WL1),
                                     t_n2[:, c0:c0 + step],
                                     start=True, stop=True)
                    evict(EV_L1[(c0 // 512) % len(EV_L1)],
                          t_l1[:, c0:c0 + step], ps[:, :step],
                          bias(BL1), True)
                for si, c0 in enumerate(steps):
                    ps = psB.tile([128, 512], f32, tag="psB")
                    nc.tensor.matmul(ps[:, :step], W(WL2),
                                     t_l1[:, c0:c0 + step],
                                     start=True, stop=True)
                    if EV_OUT == "act":
                        nc.scalar.activation(out_sb[:, c0:c0 + step],
                                             ps[:ACT, :step], Af.Identity,
                                             bias=bt[0:ACT, BL2:BL2 + 1])
                    else:
                        nc.vector.tensor_scalar_add(out_sb[:, c0:c0 + step],
                                                    ps[:ACT, :step],
                                                    bt[0:ACT, BL2:BL2 + 1])
                    nc.sync.dma_start(out=out_dram[:, c0:c0 + step],
                                      in_=out_sb[:, c0:c0 + step])

            # agg+n1 chunk c needs h2 r-blocks {2c, 2c+1} = cols up to
            # (2c+2)*QB; fire it after the e2 group covering that.
            agg_after = {}
            for c in range(5):
                agg_after[next(g for g in range(N_EG)
                               if EG0[g] + EGW[g] >= (2 * c + 2) * QB)] = c
            for g in range(N_EG):
                g0 = EG0[g]
                gw = EGW[g]
                ps = egroup_tile(g)
                for o in range(0, gw, 512):
                    nw = min(512, gw - o)
                    nc.tensor.matmul(ps[:, o:o + nw], W(WE2S),
                                     h1_t[:, g0 + o:g0 + o + nw],
                                     start=True, stop=True)
                evict(EV_H2[g], h2_t[:, g0:g0 + gw], ps[:, :gw],
                      bias(BE2S), True)
                # zero-LDWEIGHTS filler (stationary stays WE2S), gated on
                # freshly written h1 so it lands here in the PE stream
                if g + 1 < N_EG:
                    pe_filler(1, w_ap=W(WE2S), src=h1_t[:, g0:g0 + 128])
                if g in agg_after:
                    c = agg_after[g]
                    agg_chunk(c)
                    pe_filler(1, w_ap=W(WE2S), src=t_n1[:, c * 512:c * 512 + 128])
                    if c == 1:
                        slab(0, 1024, 512)
                    elif c == 3:
                        slab(1024, 1024, 512)
                    elif c == 4:
                        slab(2048, 512, 256)

    nc.compile()
    _dedupe_ldweights(nc)
    return nc


def _dedupe_ldweights(nc):
    """Remove redundant PE weight loads after bacc splits matmuls into
    Ldweights+Matmult pairs: a Ldweights whose source AP equals the
    previously loaded one (PE stream order == block order) is a no-op.
    Only drop instructions carrying no semaphore waits/updates."""
    from concourse import mybir
    import bass_rust
    for f in nc.m.functions:
        for b in f.blocks:
            last = None
            keep = []
            insts = b.instructions
            for idx, i in enumerate(insts):
                if isinstance(i, mybir.InstLdweights):
                    key = str(i.ins[0])
                    if key == last:
                        if i.sync_info is None:
                            continue
                        # migrate waits/updates onto the paired matmult so
                        # the redundant load can still be dropped
                        nxt = insts[idx + 1] if idx + 1 < len(insts) else None
                        if isinstance(nxt, mybir.InstMatmult):
                            ow = list(i.sync_info.on_wait)
                            ou = list(i.sync_info.on_update)
                            if nxt.sync_info is not None:
                                ow += list(nxt.sync_info.on_wait)
                                ou += list(nxt.sync_info.on_update)
                            if len(ow) <= 1:    # walrus: one wait per inst
                                nxt.sync_info = bass_rust.SyncInfo(
                                    on_wait=ow, on_update=ou)
                                continue
                    last = key
                keep.append(i)
            if len(keep) != len(insts):
                b.instructions[:] = keep


def _get_program():
    global _PROG
    if _PROG is None:
        _PROG = _build_program()
    return _PROG


# ------------------------------------------------------------------- kernel

def kernel(theta, s, i, senders, receivers,
           w_in1, b_in1, w_in2, b_in2,
           w_e1, b_e1, w_e2, b_e2, w_e3, b_e3,
           w_n1, b_n1, w_n2, b_n2, w_n3, b_n3,
           w_l1, b_l1, w_l2, b_l2):
    global LAST_EXEC_NS
    import os
    import ml_dtypes

    args = dict(theta=theta, s=s, i=i, senders=senders, receivers=receivers,
                w_in1=w_in1, b_in1=b_in1, w_in2=w_in2, b_in2=b_in2,
                w_e1=w_e1, b_e1=b_e1, w_e2=w_e2, b_e2=b_e2,
                w_e3=w_e3, b_e3=b_e3, w_n1=w_n1, b_n1=b_n1,
                w_n2=w_n2, b_n2=b_n2, w_n3=w_n3, b_n3=b_n3,
                w_l1=w_l1, b_l1=b_l1, w_l2=w_l2, b_l2=b_l2)
    args = {k: np.asarray(v) for k, v in args.items()}

    # The device program hardcodes the documented block-diagonal
    # fully-connected edge structure and constant-i input; verify, else
    # fall back to a host fp32 computation (correct for any input).
    exp_s, exp_r = _expected_edges()
    i_flat = np.asarray(args["i"], np.float32).reshape(-1)
    structured = (np.array_equal(np.asarray(args["senders"], np.int64), exp_s)
                  and np.array_equal(np.asarray(args["receivers"], np.int64), exp_r)
                  and np.all(i_flat == i_flat[0]))
    if not structured:
        return _fallback_numpy(**{k: np.asarray(v, np.float32)
                                  if np.asarray(v).dtype != np.int32 else np.asarray(v)
                                  for k, v in args.items()})

    f64 = np.float64
    fp8 = ml_dtypes.float8_e4m3
    w_in1_, b_in1_ = args["w_in1"].astype(f64), args["b_in1"].astype(f64)
    w_in2_, b_in2_ = args["w_in2"].astype(f64), args["b_in2"].astype(f64)
    w_e1_, b_e1_ = args["w_e1"].astype(f64), args["b_e1"].astype(f64)
    w_e3_, b_e3_ = args["w_e3"].astype(f64), args["b_e3"].astype(f64)
    w_n1_, b_n1_ = args["w_n1"].astype(f64), args["b_n1"].astype(f64)
    w_n3_, b_n3_ = args["w_n3"].astype(f64), args["b_n3"].astype(f64)
    w_l1_, b_l1_ = args["w_l1"].astype(f64), args["b_l1"].astype(f64)

    b1_eff = b_in1_ + i_flat[0] * w_in1_[H_DIM + S_DIM]
    b_u = b_e1_ + b_in2_ @ w_e1_[:MID]
    b_v = b_in2_ @ w_e1_[MID:]
    b_n1_eff = b_n1_ + b_in2_ @ w_n1_[:MID] + b_e3_ @ w_n1_[MID:]
    b_l1_eff = b_l1_ + b_n3_ @ w_l1_

    wn1b = (w_e3_ / (A - 1)) @ w_n1_[MID:]
    wslots = np.zeros((NSLOTS, 128, 128), np.float16)
    wslots[W1T] = w_in1_[:128].astype(np.float16)
    wslots[WE1T] = (w_in2_ @ w_e1_[:MID]).astype(np.float16)
    wslots[WE1B] = (w_in2_ @ w_e1_[MID:]).astype(np.float16)
    wslots[WE2S] = (args["w_e2"].astype(f64) * S_H2).astype(np.float16)
    wslots[WN1AS] = ((w_in2_ @ w_n1_[:MID]) * (S_H2 * S_WB)).astype(np.float16)
    wslots[WN2] = args["w_n2"].astype(np.float16)
    wslots[WL1] = (w_n3_ @ w_l1_).astype(np.float16)
    wslots[WL2, :, :ACT] = args["w_l2"].astype(np.float16)
    w_pack = np.ascontiguousarray(
        wslots.transpose(1, 0, 2).reshape(128, NSLOTS * 128))

    wn1b8 = np.clip(wn1b * S_WB, -240.0, 240.0).astype(fp8)
    wdr_pack = np.ascontiguousarray(
        np.concatenate([wn1b8, wn1b8, wn1b8], axis=1))          # [128, 384]

    b_pack = np.zeros((128, 8), np.float32)
    for idx, vec in ((B1, b1_eff), (BU, b_u), (BV, b_v),
                     (BE2S, args["b_e2"].astype(f64) * S_H2),
                     (BN1, b_n1_eff), (BN2, args["b_n2"]), (BL1, b_l1_eff)):
        b_pack[:, idx] = np.asarray(vec, np.float32)
    b_pack[:ACT, BL2] = args["b_l2"].astype(np.float32)

    # node features, feat-major, (a, p) column order, per-core shards
    n_all = B * P * A
    X = np.concatenate([args["theta"].reshape(n_all, H_DIM),
                        args["s"].reshape(n_all, S_DIM)], axis=-1)
    in_maps = []
    for c in range(N_CORES):
        xc = X[c * NODES:(c + 1) * NODES]
        in_maps.append({
            "x_fm": _to_ap_major(xc).astype(np.float16),
            "w_pack": w_pack,
            "wdr_pack": wdr_pack,
            "b_pack": b_pack,
        })

    nc = _get_program()
    if os.environ.get("KERNEL_SIM", "0") == "1":
        # CoreSim core 0 only (cores are identical up to data); other cores
        # return zeros. For correctness devloop, not grading.
        from concourse import bass_interp
        sim = bass_interp.CoreSim(nc)
        for k, v in in_maps[0].items():
            sim.tensor(k)[:] = v
        sim.simulate()
        results = [{"out": np.array(sim.tensor("out"))}]
        results += [{"out": np.zeros((ACT, NODES), np.float32)}
                    for _ in range(N_CORES - 1)]
        parts = [_from_ap_major(r["out"]) for r in results]
        return np.concatenate(parts, axis=0).reshape(B, P, A, ACT).astype(np.float32)

    from concourse.bass_utils import run_bass_kernel_spmd
    trace = os.environ.get("KERNEL_TRACE", "0") == "1"
    res = run_bass_kernel_spmd(nc, in_maps, core_ids=list(range(N_CORES)),
                               trace=trace)
    LAST_EXEC_NS = res.exec_time_ns

    parts = [_from_ap_major(res.results[c]["out"]) for c in range(N_CORES)]
    return np.concatenate(parts, axis=0).reshape(B, P, A, ACT).astype(np.float32)


# revision 74
# speedup vs baseline: 1.0153x; 1.0153x over previous
"""Trainium2 Bass kernel for nn_ActionPredictionNet (GNN message passing).

Data-parallel over batch*particles: 8 NeuronCores, each handling 256
independent fully-connected 10-node particle graphs (2560 nodes, 23040
edges). The fully-connected structure lets us restructure the math:

  - Edge-MLP layer 1 collapses: e_in = [n[s], n[r]] so layer-1 pre-act is
    u[s] + v[r] with u = W_top^T n, v = W_bot^T n computed per NODE
    (2560 cols) instead of per EDGE (23040 cols), then a broadcast-add.
  - Edges are only consumed via the mean over incoming messages, so edge
    layer 3 folds into the aggregation: accumulate (sum_s h2_s) @ (w_e3/9)
    in PSUM. The aggregation matmuls run in fp8 DoubleRow mode (two sender
    slots per pass), halving their PE time; h2 is stored fp8 with a 4x
    scale folded into w_e2/b_e2 and 64x into wn1b, compensated by a 1/256
    scale on the n1 eviction.
  - Diagonal (s == r) pairs are never computed: per receiver the sender
    range splits into two dense pieces.

Layouts (per core, feat-major: features on SBUF partitions):
  - node tensors [128, 2560], column = a*256 + p  (a: node-in-graph 0..9,
    p: graph 0..255)  -> broadcast APs get innermost unit stride.
  - edge tensors [128, 23040], column = r*2304 + s'*256 + p (s' skips r).

Schedule notes (from perfetto traces of the previous version):
  - input DMA is issued from three engines in parallel (scalar / sync /
    gpsimd) so the first enc matmul can start ~5us in instead of ~11us.
  - PE HAM warm-up fillers accumulate into a dedicated PSUM bank (no
    eviction sink needed); in the e2 stream they reuse the currently
    loaded stationary weights so they cost no LDWEIGHTS.
  - PSUM evictions are the bottleneck (~1.1-1.3 ns/col on ACT/DVE, PSUM
    read port is 1 elem/cycle); they are batched at FD=1280 and routed
    across ACT/DVE by tunable tables; h1 relus run on DVE (fp16 4x mode),
    optionally a few on GPSIMD.
"""

import numpy as np

B, P, A = 32, 64, 10
S_DIM, H_DIM, MID = 64, 64, 128
ACT = 8
N_CORES = 8
NP_CORE = B * P // N_CORES          # 256 particle-graphs per core
NODES = NP_CORE * A                 # 2560 nodes per core
QB = (A - 1) * NP_CORE              # 2304 edge columns per receiver block
ECOLS = A * QB                      # 23040 (r, s', p) edge columns per core

GW = 1024                           # eviction group width (2 PSUM banks)
N_EG = (ECOLS + GW - 1) // GW       # 23 edge groups (last one 512 wide)

# fp8 scaling for the aggregation path
S_H2 = 4.0                          # h2 stored as 4*h2 (folded into w_e2/b_e2)
S_WB = 64.0                         # wn1b stored as 64*wn1b
S_N1 = 1.0 / (S_H2 * S_WB)          # eviction scale on the n1 pre-act
S_W1 = 16.0                         # w_in1 stored fp8 as 16*w_in1

_PROG = None        # cached compiled program: (nc, meta)
LAST_EXEC_NS = None  # filled when KERNEL_TRACE=1


# ------------------------------------------------------------ tuning tables
# eviction engine per group: enc(2), u(2), v(2), h2(18); relu engine per r
EV_ENC = ["act", "vec", "act"]
EV_U = ["vec", "act", "vec"]
EV_V = ["act", "vec", "vec"]
EV_H2 = ["act"] * 20 + ["act", "vec", "act"]
RELU_ENG = ["act", "vec", "vec", "vec", "vec", "vec",
            "vec", "vec", "vec", "vec"]  # per receiver block
EV_N2 = "vec"
EV_L1 = ["vec", "vec", "vec", "vec", "vec"]   # per 512-col slab step
EV_OUT = "vec"


# ---------------------------------------------------------------- host utils

def _expected_edges():
    a = np.arange(A)
    s, r = np.meshgrid(a, a, indexing="ij")
    m = s != r
    s, r = s[m], r[m]
    offs = (np.arange(B * P) * A)[:, None]
    return (offs + s[None, :]).reshape(-1).astype(np.int64), \
           (offs + r[None, :]).reshape(-1).astype(np.int64)


def _to_ap_major(x_core):
    """[2560, D] in (p, a) node order -> [D, 2560] feat-major, (a, p) cols."""
    return np.ascontiguousarray(
        x_core.reshape(NP_CORE, A, -1).transpose(1, 0, 2).reshape(NODES, -1).T
    )


def _from_ap_major(out_core):
    """[ACT, 2560] feat-major (a, p) cols -> [2560, ACT] in (p, a) order."""
    return out_core.T.reshape(A, NP_CORE, ACT).transpose(1, 0, 2).reshape(NODES, ACT)


def _fallback_numpy(theta, s, i, senders, receivers,
                    w_in1, b_in1, w_in2, b_in2,
                    w_e1, b_e1, w_e2, b_e2, w_e3, b_e3,
                    w_n1, b_n1, w_n2, b_n2, w_n3, b_n3,
                    w_l1, b_l1, w_l2, b_l2):
    """fp32 numpy replica of the reference; used only if inputs deviate from
    the documented structure (non-fully-connected edges or non-constant i)."""
    N = B * P * A
    relu = lambda x: np.maximum(x, 0.0)
    x = np.concatenate([theta.reshape(N, H_DIM), s.reshape(N, S_DIM),
                        i.reshape(N, 1)], axis=-1).astype(np.float32)
    n = relu(x @ w_in1 + b_in1) @ w_in2 + b_in2
    e_in = np.concatenate([n[senders], n[receivers]], axis=-1)
    e = relu(e_in @ w_e1 + b_e1)
    e = relu(e @ w_e2 + b_e2)
    e = e @ w_e3 + b_e3
    agg = np.zeros((N, e.shape[1]), np.float32)
    np.add.at(agg, receivers, e)
    agg /= (A - 1)
    h = np.concatenate([n, agg], axis=-1)
    h = relu(h @ w_n1 + b_n1)
    h = relu(h @ w_n2 + b_n2)
    h = h @ w_n3 + b_n3
    out = relu(h @ w_l1 + b_l1) @ w_l2 + b_l2
    return out.reshape(B, P, A, ACT).astype(np.float32)


# ------------------------------------------------------------- device program

# fp16 weight-pack slot indices (linear-linear layer pairs folded on host:
# w_in2 into we1t/we1b/wn1a, w_n3 into w_l1; w_e3/9 lives in the fp8 pack)
W1T, WE1T, WE1B, WE2S, WN1AS, WN2, WL1, WL2 = range(8)
NSLOTS = 8
# bias-pack column indices
B1, BU, BV, BE2S, BN1, BN2, BL1, BL2 = range(8)


def _build_program():
    import concourse.bass as bass
    import concourse.mybir as mybir
    import concourse.tile as tile
    from concourse import bacc

    f16 = mybir.dt.float16
    f32 = mybir.dt.float32
    f8 = mybir.dt.float8e4
    Af = mybir.ActivationFunctionType
    Op = mybir.AluOpType
    DR = mybir.MatmulPerfMode.DoubleRow

    nc = bacc.Bacc("TRN2", target_bir_lowering=False, debug=False)
    x_dram = nc.dram_tensor("x_fm", [128, NODES], f8, kind="ExternalInput").ap()
    w_dram = nc.dram_tensor("w_pack", [128, NSLOTS * 128], f16,
                            kind="ExternalInput").ap()
    wdr_dram = nc.dram_tensor("wdr_pack", [128, 512], f8,
                              kind="ExternalInput").ap()
    b_dram = nc.dram_tensor("b_pack", [128, 8], f32, kind="ExternalInput").ap()
    out_dram = nc.dram_tensor("out", [ACT, NODES], f32, kind="ExternalOutput").ap()

    with tile.TileContext(nc) as tc:
        with (
            tc.tile_pool(name="consts", bufs=1) as consts,
            tc.tile_pool(name="bigs", bufs=1) as bigs,
            tc.tile_pool(name="psA", bufs=2, space="PSUM") as psA,
            tc.tile_pool(name="psB", bufs=3, space="PSUM") as psB,
            tc.tile_pool(name="psF", bufs=1, space="PSUM") as psF,
        ):
            wt = consts.tile([128, NSLOTS * 128], f16, tag="wt")
            wdr = consts.tile([128, 512], f8, tag="wdr")
            bt = consts.tile([128, 8], f32, tag="bt")
            x_fm = bigs.tile([128, NODES], f8, tag="x_fm")
            dummy = consts.tile([128, 256], f16, tag="dummy")
            dsink = consts.tile([128, 8], f32, tag="dsink")

            # ---- input DMA, spread across three issuing engines so the
            # transfers overlap; first enc group only needs x[:, :1280].
            nc.gpsimd.memset(dummy[:], 0.0)
            nc.gpsimd.dma_start(out=x_fm[:, 1920:2560], in_=x_dram[:, 1920:2560])
            nc.scalar.dma_start(out=x_fm[:, 0:1280], in_=x_dram[:, 0:1280])
            nc.scalar.dma_start(out=bt[:], in_=b_dram)
            nc.sync.dma_start(out=wt[:, :3 * 128], in_=w_dram[:, :3 * 128])
            nc.sync.dma_start(out=x_fm[:, 1280:1920], in_=x_dram[:, 1280:1920])
            nc.sync.dma_start(out=wt[:, 3 * 128:], in_=w_dram[:, 3 * 128:])
            nc.sync.dma_start(out=wdr[:], in_=wdr_dram)

            # touch the Relu table set early so ACT_TABLE_LOAD hides in the
            # DMA-wait head instead of stalling the first real eviction
            nc.scalar.activation(dsink[:, 0:1], dummy[:, 0:2].bitcast(f32),
                                 Af.Relu)

            W = lambda k: wt[:, k * 128:(k + 1) * 128]
            bias = lambda k: bt[:, k:k + 1]
            wdr_pair = wdr[:, 0:256].rearrange("f (j m) -> f j m", j=2)
            wdr_one = wdr[:, 256:384]

            # ---- HAM warm-up fillers: accumulate into a dedicated PSUM
            # bank, no eviction needed. `w_ap`/`src` choose the stationary /
            # moving operands: reusing the neighbouring real matmuls'
            # stationary makes a filler cost zero LDWEIGHTS; reading freshly
            # produced data pins the filler to that point of the pipeline.
            fps = psF.tile([128, 512], f32, tag="psF")

            def pe_filler(n=1, w_ap=None, src=None):
                mv = src if src is not None else dummy[:, :256]
                st = w_ap if w_ap is not None else dummy[:, :128]
                fd = mv.shape[-1]
                for _ in range(n):
                    nc.tensor.matmul(fps[:, :fd], st, mv,
                                     start=True, stop=True,
                                     skip_group_check=True)

            pe_filler(22)  # warm-up bridging the whole input-DMA wait:
                           # PE must be at 2.4 GHz when x lands (~10us)

            t_enc = bigs.tile([128, NODES], f16, tag="t_enc")
            u_t = bigs.tile([128, NODES], f16, tag="u_t")
            v_t = bigs.tile([128, NODES], f16, tag="v_t")
            h1_t = bigs.tile([128, ECOLS], f16, tag="h1_t")
            h2_t = bigs.tile([128, ECOLS], f8, tag="h2_t")
            t_n1 = bigs.tile([128, NODES], f16, tag="t_n1")
            t_n2 = bigs.tile([128, NODES], f16, tag="t_n2")
            t_l1 = bigs.tile([128, NODES], f16, tag="t_l1")
            out_sb = bigs.tile([ACT, NODES], f32, tag="out_sb")

            def evict(eng, dst, src, bias_ap, relu, scale=None):
                if eng == "act":
                    if scale is not None:
                        nc.scalar.activation(dst, src, Af.Relu if relu else
                                             Af.Identity, bias=bias_ap,
                                             scale=scale)
                    elif relu:
                        nc.scalar.activation(dst, src, Af.Relu, bias=bias_ap)
                    elif bias_ap is not None:
                        nc.scalar.activation(dst, src, Af.Identity,
                                             bias=bias_ap)
                    else:
                        nc.scalar.copy(dst, src)
                else:
                    assert scale is None
                    if relu:
                        nc.vector.tensor_scalar(dst, src, bias_ap, 0.0,
                                                Op.add, Op.max)
                    elif bias_ap is not None:
                        nc.vector.tensor_scalar_add(dst, src, bias_ap)
                    else:
                        nc.vector.tensor_copy(dst, src)

            def node_layer(w_ap, src, dst, bias_idx, relu, engines,
                           scale=None):
                """2560-col dense layer as FD<=1024 PSUM groups."""
                for gi, g0 in enumerate(range(0, NODES, GW)):
                    gw = min(GW, NODES - g0)
                    ps = psA.tile([128, GW], f32, tag="psA")
                    for o in range(0, gw, 512):
                        nw = min(512, gw - o)
                        nc.tensor.matmul(ps[:, o:o + nw], w_ap,
                                         src[:, g0 + o:g0 + o + nw],
                                         start=True, stop=True)
                    evict(engines[gi], dst[:, g0:g0 + gw], ps[:, :gw],
                          bias(bias_idx), relu, scale=scale)

            # ---- node encoder + edge layer-1 node halves (w_in2 folded in).
            # The encoder runs fp8 (x and 16*w_in1); t_enc is stored as
            # 16*t_enc (bias pre-scaled on host, consumers' weights /16) so
            # its evictions stay engine-agnostic. Groups are interleaved so
            # the adds' gating set {u g0, u g1, v g0} evicts earliest.
            def head_group(w_ap, src, dst, bias_idx, relu, eng, g0):
                gw = min(GW, NODES - g0)
                ps = psA.tile([128, GW], f32, tag="psA")
                for o in range(0, gw, 512):
                    nw = min(512, gw - o)
                    nc.tensor.matmul(ps[:, o:o + nw], w_ap,
                                     src[:, g0 + o:g0 + o + nw],
                                     start=True, stop=True)
                evict(eng, dst[:, g0:g0 + gw], ps[:, :gw],
                      bias(bias_idx), relu)

            w1 = wdr[:, 384:512]
            head_group(w1, x_fm, t_enc, B1, True, "act", 0)
            head_group(w1, x_fm, t_enc, B1, True, "vec", 1024)
            head_group(w1, x_fm, t_enc, B1, True, "act", 2048)
            pe_filler(1, src=t_enc[:, 0:256])
            head_group(W(WE1T), t_enc, u_t, BU, False, "vec", 0)
            head_group(W(WE1B), t_enc, v_t, BV, False, "act", 0)
            pe_filler(1, src=u_t[:, 0:256])
            head_group(W(WE1T), t_enc, u_t, BU, False, "vec", 1024)
            head_group(W(WE1T), t_enc, u_t, BU, False, "act", 2048)
            pe_filler(1, src=v_t[:, 0:256])
            # v g1/g2 are only needed by adds r4 / r8 — issued inside the
            # r-loop below so they don't sit ahead of the first adds in
            # DVE's FIFO

            # ---- h1 = relu(u[s] + v[r]) over (r, s', p) columns, where the
            # 9 sender slots s' skip s == r (no diagonal is ever computed).
            v3 = v_t[:].rearrange("f (r p) -> f r p", p=NP_CORE)

            def tt_r(r):
                w0 = r * QB
                vb1 = v3[:, r:r + 1, :]
                ranges = [(0, r, w0), (r + 1, A, w0 + r * NP_CORE)]
                if r < 2:
                    # split the long range at the u-eviction-group boundary
                    # (s-block 8 = u col 2048) so the first piece only waits
                    # on u groups 0-1 and the adds start ~1us earlier
                    lo, hi, d0 = ranges.pop()
                    ranges += [(lo, 8, d0), (8, hi, d0 + (8 - lo) * NP_CORE)]
                first = True
                for lo, hi, d0 in ranges:
                    k = hi - lo
                    if k == 0:
                        continue
                    o = h1_t[:, d0:d0 + k * NP_CORE] \
                        .rearrange("f (s p) -> f s p", p=NP_CORE)
                    us = u_t[:, lo * NP_CORE:hi * NP_CORE] \
                        .rearrange("f (s p) -> f s p", p=NP_CORE)
                    nc.vector.tensor_add(o, us,
                                         vb1.broadcast_to([128, k, NP_CORE]))
                    if r < 2 and first:
                        pe_filler(1, w_ap=W(WE2S),
                                  src=h1_t[:, d0:d0 + 128])
                    first = False

                def relu_part(flat):
                    eng = RELU_ENG[r]
                    if eng == "act":
                        nc.scalar.activation(flat, flat, Af.Relu)
                    else:
                        nc.vector.tensor_scalar_max(flat, flat, 0.0)
                if r < 2:
                    # split so the first e2 groups unblock sooner
                    relu_part(h1_t[:, w0:w0 + GW])
                    relu_part(h1_t[:, w0 + GW:w0 + QB])
                else:
                    relu_part(h1_t[:, w0:w0 + QB])

            pe_filler(1, src=u_t[:, 2304:2560])
            for r in range(A):
                tt_r(r)
                if r == 1:
                    head_group(W(WE1B), t_enc, v_t, BV, False, "vec", 1024)
                elif r == 3:
                    head_group(W(WE1B), t_enc, v_t, BV, False, "vec", 2048)
                if r < 3:
                    pe_filler(2, src=h1_t[:, r * QB:r * QB + 256])

            # ---- main stream: h2 = relu(w_e2s^T h1 + b_e2s) stored fp8;
            # after the groups covering r-blocks {2c, 2c+1}, the fused
            # agg+n1 chunk runs (fp8 DoubleRow over sender-slot pairs), and
            # per completed t_n1 slab the rest of the network.
            h2v = h2_t[:].rearrange("f (r s p) -> f s r p", s=A - 1,
                                    p=NP_CORE)

            def agg_chunk(c):
                ps = psB.tile([128, 512], f32, tag="psB")
                nc.tensor.matmul(ps[:], W(WN1AS),
                                 t_enc[:, c * 512:(c + 1) * 512],
                                 start=True, stop=False)
                for ri, r in enumerate((2 * c, 2 * c + 1)):
                    po = ri * NP_CORE
                    for a2 in range(4):
                        nc.tensor.matmul(
                            ps[:, po:po + NP_CORE], wdr_pair,
                            h2v[:, 2 * a2:2 * a2 + 2, r:r + 1, :],
                            start=False, stop=False, perf_mode=DR)
                    nc.tensor.matmul(ps[:, po:po + NP_CORE], wdr_one,
                                     h2v[:, 8:9, r:r + 1, :],
                                     start=False, stop=(ri == 1))
                # t_n1 stored at 256x (bias pre-scaled, wn2 slot /256 on
                # the host) so this eviction is engine-agnostic
                evict("vec" if c == 4 else "act",
                      t_n1[:, c * 512:(c + 1) * 512], ps[:],
                      bias(BN1), True)

            def slab(s0, sw, step):
                """node-MLP tail + decoder for t_n1 cols [s0, s0+sw).
                Stage-major over the slab so each weight loads once."""
                steps = list(range(s0, s0 + sw, step))
                tail = s0 >= 2048   # ACT's h2 stream is done; use it for
                                    # latency on the final chain
                for si, c0 in enumerate(steps):
                    ps = psB.tile([128, 512], f32, tag="psB")
                    nc.tensor.matmul(ps[:, :step], W(WN2),
                                     t_n1[:, c0:c0 + step],
                                     start=True, stop=True)
                    evict(("act", "vec")[si % 2] if tail else EV_N2,
                          t_n2[:, c0:c0 + step], ps[:, :step],
                          bias(BN2), True)
                for si, c0 in enumerate(steps):
                    ps = psB.tile([128, 512], f32, tag="psB")
                    nc.tensor.matmul(ps[:, :step], W(WL1),
                                     t_n2[:, c0:c0 + step],
                                     start=True, stop=True)
                    evict(("vec", "act")[si % 2] if tail else
                          EV_L1[(c0 // 512) % len(EV_L1)],
                          t_l1[:, c0:c0 + step], ps[:, :step],
                          bias(BL1), True)
                for si, c0 in enumerate(steps):
                    ps = psB.tile([128, 512], f32, tag="psB")
                    nc.tensor.matmul(ps[:, :step], W(WL2),
                                     t_l1[:, c0:c0 + step],
                                     start=True, stop=True)
                    if (("act", "vec")[si % 2] if tail else EV_OUT) == "act":
                        nc.scalar.activation(out_sb[:, c0:c0 + step],
                                             ps[:ACT, :step], Af.Identity,
                                             bias=bt[0:ACT, BL2:BL2 + 1])
                    else:
                        nc.vector.tensor_scalar_add(out_sb[:, c0:c0 + step],
                                                    ps[:ACT, :step],
                                                    bt[0:ACT, BL2:BL2 + 1])
                    nc.sync.dma_start(out=out_dram[:, c0:c0 + step],
                                      in_=out_sb[:, c0:c0 + step])

            # agg+n1 chunk c needs h2 r-blocks {2c, 2c+1} = cols up to
            # (2c+2)*QB; fire it after the GW-col group covering that.
            agg_after = {((2 * c + 2) * QB - 1) // GW: c for c in range(5)}
            for g in range(N_EG):
                g0 = g * GW
                gw = min(GW, ECOLS - g0)
                ps = psA.tile([128, GW], f32, tag="psA")
                for o in range(0, gw, 512):
                    nw = min(512, gw - o)
                    nc.tensor.matmul(ps[:, o:o + nw], W(WE2S),
                                     h1_t[:, g0 + o:g0 + o + nw],
                                     start=True, stop=True)
                evict(EV_H2[g], h2_t[:, g0:g0 + gw], ps[:, :gw],
                      bias(BE2S), True)
                # zero-LDWEIGHTS filler (stationary stays WE2S), gated on
                # freshly written h1 so it lands here in the PE stream
                if g + 1 < N_EG:
                    pe_filler(1, w_ap=W(WE2S), src=h1_t[:, g0:g0 + 128])
                if g in agg_after:
                    c = agg_after[g]
                    agg_chunk(c)
                    pe_filler(1, w_ap=W(WE2S), src=t_n1[:, c * 512:c * 512 + 128])
                    if c == 1:
                        slab(0, 1024, 512)
                    elif c == 3:
                        slab(1024, 1024, 512)
                    elif c == 4:
                        slab(2048, 512, 256)

    nc.compile()
    _dedupe_ldweights(nc)
    return nc


def _dedupe_ldweights(nc):
    """Remove redundant PE weight loads after bacc splits matmuls into
    Ldweights+Matmult pairs: a Ldweights whose source AP equals the
    previously loaded one (PE stream order == block order) is a no-op.
    Only drop instructions carrying no semaphore waits/updates."""
    from concourse import mybir
    import bass_rust
    for f in nc.m.functions:
        for b in f.blocks:
            last = None
            keep = []
            insts = b.instructions
            for idx, i in enumerate(insts):
                if isinstance(i, mybir.InstLdweights):
                    key = str(i.ins[0])
                    if key == last:
                        if i.sync_info is None:
                            continue
                        # migrate waits/updates onto the paired matmult so
                        # the redundant load can still be dropped
                        nxt = insts[idx + 1] if idx + 1 < len(insts) else None
                        if isinstance(nxt, mybir.InstMatmult):
                            ow = list(i.sync_info.on_wait)
                            ou = list(i.sync_info.on_update)
                            if nxt.sync_info is not None:
                                ow += list(nxt.sync_info.on_wait)
                                ou += list(nxt.sync_info.on_update)
                            if len(ow) <= 1:    # walrus: one wait per inst
                                nxt.sync_info = bass_rust.SyncInfo(
                                    on_wait=ow, on_update=ou)
                                continue
                    last = key
                keep.append(i)
            if len(keep) != len(insts):
                b.instructions[:] = keep


def _get_program():
    global _PROG
    if _PROG is None:
        _PROG = _build_program()
    return _PROG


# ------------------------------------------------------------------- kernel

def kernel(theta, s, i, senders, receivers,
           w_in1, b_in1, w_in2, b_in2,
           w_e1, b_e1, w_e2, b_e2, w_e3, b_e3,
           w_n1, b_n1, w_n2, b_n2, w_n3, b_n3,
           w_l1, b_l1, w_l2, b_l2):
    global LAST_EXEC_NS
    import os
    import ml_dtypes

    args = dict(theta=theta, s=s, i=i, senders=senders, receivers=receivers,
                w_in1=w_in1, b_in1=b_in1, w_in2=w_in2, b_in2=b_in2,
                w_e1=w_e1, b_e1=b_e1, w_e2=w_e2, b_e2=b_e2,
                w_e3=w_e3, b_e3=b_e3, w_n1=w_n1, b_n1=b_n1,
                w_n2=w_n2, b_n2=b_n2, w_n3=w_n3, b_n3=b_n3,
                w_l1=w_l1, b_l1=b_l1, w_l2=w_l2, b_l2=b_l2)
    args = {k: np.asarray(v) for k, v in args.items()}

    # The device program hardcodes the documented block-diagonal
    # fully-connected edge structure and constant-i input; verify, else
    # fall back to a host fp32 computation (correct for any input).
    exp_s, exp_r = _expected_edges()
    i_flat = np.asarray(args["i"], np.float32).reshape(-1)
    structured = (np.array_equal(np.asarray(args["senders"], np.int64), exp_s)
                  and np.array_equal(np.asarray(args["receivers"], np.int64), exp_r)
                  and np.all(i_flat == i_flat[0]))
    if not structured:
        return _fallback_numpy(**{k: np.asarray(v, np.float32)
                                  if np.asarray(v).dtype != np.int32 else np.asarray(v)
                                  for k, v in args.items()})

    f64 = np.float64
    fp8 = ml_dtypes.float8_e4m3
    w_in1_, b_in1_ = args["w_in1"].astype(f64), args["b_in1"].astype(f64)
    w_in2_, b_in2_ = args["w_in2"].astype(f64), args["b_in2"].astype(f64)
    w_e1_, b_e1_ = args["w_e1"].astype(f64), args["b_e1"].astype(f64)
    w_e3_, b_e3_ = args["w_e3"].astype(f64), args["b_e3"].astype(f64)
    w_n1_, b_n1_ = args["w_n1"].astype(f64), args["b_n1"].astype(f64)
    w_n3_, b_n3_ = args["w_n3"].astype(f64), args["b_n3"].astype(f64)
    w_l1_, b_l1_ = args["w_l1"].astype(f64), args["b_l1"].astype(f64)

    b1_eff = (b_in1_ + i_flat[0] * w_in1_[H_DIM + S_DIM]) * S_W1
    b_u = b_e1_ + b_in2_ @ w_e1_[:MID]
    b_v = b_in2_ @ w_e1_[MID:]
    b_n1_eff = (b_n1_ + b_in2_ @ w_n1_[:MID] + b_e3_ @ w_n1_[MID:]) \
        * (S_H2 * S_WB)
    b_l1_eff = b_l1_ + b_n3_ @ w_l1_

    wn1b = (w_e3_ / (A - 1)) @ w_n1_[MID:]
    wslots = np.zeros((NSLOTS, 128, 128), np.float16)
    wslots[W1T] = w_in1_[:128].astype(np.float16)
    wslots[WE1T] = (w_in2_ @ w_e1_[:MID] / S_W1).astype(np.float16)
    wslots[WE1B] = (w_in2_ @ w_e1_[MID:] / S_W1).astype(np.float16)
    wslots[WE2S] = (args["w_e2"].astype(f64) * S_H2).astype(np.float16)
    wslots[WN1AS] = ((w_in2_ @ w_n1_[:MID]) * (S_H2 * S_WB / S_W1)
                     ).astype(np.float16)
    wslots[WN2] = (args["w_n2"].astype(f64) / (S_H2 * S_WB)
                   ).astype(np.float16)
    wslots[WL1] = (w_n3_ @ w_l1_).astype(np.float16)
    wslots[WL2, :, :ACT] = args["w_l2"].astype(np.float16)
    w_pack = np.ascontiguousarray(
        wslots.transpose(1, 0, 2).reshape(128, NSLOTS * 128))

    wn1b8 = np.clip(wn1b * S_WB, -240.0, 240.0).astype(fp8)
    w18 = np.clip(w_in1_[:128] * S_W1, -240.0, 240.0).astype(fp8)
    wdr_pack = np.ascontiguousarray(
        np.concatenate([wn1b8, wn1b8, wn1b8, w18], axis=1))     # [128, 512]

    b_pack = np.zeros((128, 8), np.float32)
    for idx, vec in ((B1, b1_eff), (BU, b_u), (BV, b_v),
                     (BE2S, args["b_e2"].astype(f64) * S_H2),
                     (BN1, b_n1_eff), (BN2, args["b_n2"]), (BL1, b_l1_eff)):
        b_pack[:, idx] = np.asarray(vec, np.float32)
    b_pack[:ACT, BL2] = args["b_l2"].astype(np.float32)

    # node features, feat-major, (a, p) column order, per-core shards
    n_all = B * P * A
    X = np.concatenate([args["theta"].reshape(n_all, H_DIM),
                        args["s"].reshape(n_all, S_DIM)], axis=-1)
    in_maps = []
    for c in range(N_CORES):
        xc = X[c * NODES:(c + 1) * NODES]
        in_maps.append({
            "x_fm": np.clip(_to_ap_major(xc), -240.0, 240.0).astype(fp8),
            "w_pack": w_pack,
            "wdr_pack": wdr_pack,
            "b_pack": b_pack,
        })

    nc = _get_program()
    if os.environ.get("KERNEL_SIM", "0") == "1":
        # CoreSim core 0 only (cores are identical up to data); other cores
        # return zeros. For correctness devloop, not grading.
        from concourse import bass_interp
        sim = bass_interp.CoreSim(nc)
        for k, v in in_maps[0].items():
            sim.tensor(k)[:] = v
        sim.simulate()
        results = [{"out": np.array(sim.tensor("out"))}]
        results += [{"out": np.zeros((ACT, NODES), np.float32)}
                    for _ in range(N_CORES - 1)]
        parts = [_from_ap_major(r["out"]) for r in results]
        return np.concatenate(parts, axis=0).reshape(B, P, A, ACT).astype(np.float32)

    from concourse.bass_utils import run_bass_kernel_spmd
    trace = os.environ.get("KERNEL_TRACE", "0") == "1"
    res = run_bass_kernel_spmd(nc, in_maps, core_ids=list(range(N_CORES)),
                               trace=trace)
    LAST_EXEC_NS = res.exec_time_ns

    parts = [_from_ap_major(res.results[c]["out"]) for c in range(N_CORES)]
    return np.concatenate(parts, axis=0).reshape(B, P, A, ACT).astype(np.float32)


# revision 75
# speedup vs baseline: 1.0393x; 1.0236x over previous
"""Trainium2 Bass kernel for nn_ActionPredictionNet (GNN message passing).

Data-parallel over batch*particles: 8 NeuronCores, each handling 256
independent fully-connected 10-node particle graphs (2560 nodes, 23040
edges). The fully-connected structure lets us restructure the math:

  - Edge-MLP layer 1 collapses: e_in = [n[s], n[r]] so layer-1 pre-act is
    u[s] + v[r] with u = W_top^T n, v = W_bot^T n computed per NODE
    (2560 cols) instead of per EDGE (23040 cols), then a broadcast-add.
  - Edges are only consumed via the mean over incoming messages, so edge
    layer 3 folds into the aggregation: accumulate (sum_s h2_s) @ (w_e3/9)
    in PSUM. The aggregation matmuls run in fp8 DoubleRow mode (two sender
    slots per pass), halving their PE time; h2 is stored fp8 with a 4x
    scale folded into w_e2/b_e2 and 64x into wn1b, compensated by a 1/256
    scale on the n1 eviction.
  - Diagonal (s == r) pairs are never computed: per receiver the sender
    range splits into two dense pieces.

Layouts (per core, feat-major: features on SBUF partitions):
  - node tensors [128, 2560], column = a*256 + p  (a: node-in-graph 0..9,
    p: graph 0..255)  -> broadcast APs get innermost unit stride.
  - edge tensors [128, 23040], column = r*2304 + s'*256 + p (s' skips r).

Schedule notes (from perfetto traces of the previous version):
  - input DMA is issued from three engines in parallel (scalar / sync /
    gpsimd) so the first enc matmul can start ~5us in instead of ~11us.
  - PE HAM warm-up fillers accumulate into a dedicated PSUM bank (no
    eviction sink needed); in the e2 stream they reuse the currently
    loaded stationary weights so they cost no LDWEIGHTS.
  - PSUM evictions are the bottleneck (~1.1-1.3 ns/col on ACT/DVE, PSUM
    read port is 1 elem/cycle); they are batched at FD=1280 and routed
    across ACT/DVE by tunable tables; h1 relus run on DVE (fp16 4x mode),
    optionally a few on GPSIMD.
"""

import numpy as np

B, P, A = 32, 64, 10
S_DIM, H_DIM, MID = 64, 64, 128
ACT = 8
N_CORES = 8
NP_CORE = B * P // N_CORES          # 256 particle-graphs per core
NODES = NP_CORE * A                 # 2560 nodes per core
QB = (A - 1) * NP_CORE              # 2304 edge columns per receiver block
ECOLS = A * QB                      # 23040 (r, s', p) edge columns per core

GW = 1024                           # eviction group width (2 PSUM banks)
N_EG = (ECOLS + GW - 1) // GW       # 23 edge groups (last one 512 wide)

# fp8 scaling for the aggregation path
S_H2 = 4.0                          # h2 stored as 4*h2 (folded into w_e2/b_e2)
S_WB = 64.0                         # wn1b stored as 64*wn1b
S_N1 = 1.0 / (S_H2 * S_WB)          # eviction scale on the n1 pre-act
S_W1 = 16.0                         # w_in1 stored fp8 as 16*w_in1

_PROG = None        # cached compiled program: (nc, meta)
LAST_EXEC_NS = None  # filled when KERNEL_TRACE=1


# ------------------------------------------------------------ tuning tables
# eviction engine per group: enc(2), u(2), v(2), h2(18); relu engine per r
EV_ENC = ["act", "vec", "act"]
EV_U = ["vec", "act", "vec"]
EV_V = ["act", "vec", "vec"]
EV_H2 = ["act"] * 19 + ["vec"] + ["act", "vec", "act"]
RELU_ENG = ["act", "vec", "vec", "vec", "vec", "vec",
            "vec", "vec", "vec", "vec"]  # per receiver block
EV_N2 = "vec"
EV_L1 = ["vec", "vec", "vec", "vec", "vec"]   # per 512-col slab step
EV_OUT = "vec"


# ---------------------------------------------------------------- host utils

def _expected_edges():
    a = np.arange(A)
    s, r = np.meshgrid(a, a, indexing="ij")
    m = s != r
    s, r = s[m], r[m]
    offs = (np.arange(B * P) * A)[:, None]
    return (offs + s[None, :]).reshape(-1).astype(np.int64), \
           (offs + r[None, :]).reshape(-1).astype(np.int64)


def _to_ap_major(x_core):
    """[2560, D] in (p, a) node order -> [D, 2560] feat-major, (a, p) cols."""
    return np.ascontiguousarray(
        x_core.reshape(NP_CORE, A, -1).transpose(1, 0, 2).reshape(NODES, -1).T
    )


def _from_ap_major(out_core):
    """[ACT, 2560] feat-major (a, p) cols -> [2560, ACT] in (p, a) order."""
    return out_core.T.reshape(A, NP_CORE, ACT).transpose(1, 0, 2).reshape(NODES, ACT)


def _fallback_numpy(theta, s, i, senders, receivers,
                    w_in1, b_in1, w_in2, b_in2,
                    w_e1, b_e1, w_e2, b_e2, w_e3, b_e3,
                    w_n1, b_n1, w_n2, b_n2, w_n3, b_n3,
                    w_l1, b_l1, w_l2, b_l2):
    """fp32 numpy replica of the reference; used only if inputs deviate from
    the documented structure (non-fully-connected edges or non-constant i)."""
    N = B * P * A
    relu = lambda x: np.maximum(x, 0.0)
    x = np.concatenate([theta.reshape(N, H_DIM), s.reshape(N, S_DIM),
                        i.reshape(N, 1)], axis=-1).astype(np.float32)
    n = relu(x @ w_in1 + b_in1) @ w_in2 + b_in2
    e_in = np.concatenate([n[senders], n[receivers]], axis=-1)
    e = relu(e_in @ w_e1 + b_e1)
    e = relu(e @ w_e2 + b_e2)
    e = e @ w_e3 + b_e3
    agg = np.zeros((N, e.shape[1]), np.float32)
    np.add.at(agg, receivers, e)
    agg /= (A - 1)
    h = np.concatenate([n, agg], axis=-1)
    h = relu(h @ w_n1 + b_n1)
    h = relu(h @ w_n2 + b_n2)
    h = h @ w_n3 + b_n3
    out = relu(h @ w_l1 + b_l1) @ w_l2 + b_l2
    return out.reshape(B, P, A, ACT).astype(np.float32)


# ------------------------------------------------------------- device program

# fp16 weight-pack slot indices (linear-linear layer pairs folded on host:
# w_in2 into we1t/we1b/wn1a, w_n3 into w_l1; w_e3/9 lives in the fp8 pack)
W1T, WE1T, WE1B, WE2S, WN1AS, WN2, WL1, WL2 = range(8)
NSLOTS = 8
# bias-pack column indices
B1, BU, BV, BE2S, BN1, BN2, BL1, BL2 = range(8)


def _build_program():
    import concourse.bass as bass
    import concourse.mybir as mybir
    import concourse.tile as tile
    from concourse import bacc

    f16 = mybir.dt.float16
    f32 = mybir.dt.float32
    f8 = mybir.dt.float8e4
    Af = mybir.ActivationFunctionType
    Op = mybir.AluOpType
    DR = mybir.MatmulPerfMode.DoubleRow

    nc = bacc.Bacc("TRN2", target_bir_lowering=False, debug=False)
    x_dram = nc.dram_tensor("x_fm", [128, NODES], f16, kind="ExternalInput").ap()
    w_dram = nc.dram_tensor("w_pack", [128, NSLOTS * 128], f16,
                            kind="ExternalInput").ap()
    wdr_dram = nc.dram_tensor("wdr_pack", [128, 384], f8,
                              kind="ExternalInput").ap()
    b_dram = nc.dram_tensor("b_pack", [128, 8], f32, kind="ExternalInput").ap()
    out_dram = nc.dram_tensor("out", [ACT, NODES], f32, kind="ExternalOutput").ap()

    with tile.TileContext(nc) as tc:
        with (
            tc.tile_pool(name="consts", bufs=1) as consts,
            tc.tile_pool(name="bigs", bufs=1) as bigs,
            tc.tile_pool(name="psA", bufs=2, space="PSUM") as psA,
            tc.tile_pool(name="psB", bufs=3, space="PSUM") as psB,
            tc.tile_pool(name="psF", bufs=1, space="PSUM") as psF,
        ):
            wt = consts.tile([128, NSLOTS * 128], f16, tag="wt")
            wdr = consts.tile([128, 384], f8, tag="wdr")
            bt = consts.tile([128, 8], f32, tag="bt")
            x_fm = bigs.tile([128, NODES], f16, tag="x_fm")
            dummy = consts.tile([128, 256], f16, tag="dummy")
            dsink = consts.tile([128, 8], f32, tag="dsink")

            # ---- input DMA, spread across three issuing engines so the
            # transfers overlap; first enc group only needs x[:, :1280].
            nc.gpsimd.memset(dummy[:], 0.0)
            nc.gpsimd.dma_start(out=x_fm[:, 1920:2560], in_=x_dram[:, 1920:2560])
            nc.scalar.dma_start(out=x_fm[:, 0:1280], in_=x_dram[:, 0:1280])
            nc.scalar.dma_start(out=bt[:], in_=b_dram)
            nc.sync.dma_start(out=wt[:, :3 * 128], in_=w_dram[:, :3 * 128])
            nc.sync.dma_start(out=x_fm[:, 1280:1920], in_=x_dram[:, 1280:1920])
            nc.sync.dma_start(out=wt[:, 3 * 128:], in_=w_dram[:, 3 * 128:])
            nc.sync.dma_start(out=wdr[:], in_=wdr_dram)

            # touch the Relu table set early so ACT_TABLE_LOAD hides in the
            # DMA-wait head instead of stalling the first real eviction
            nc.scalar.activation(dsink[:, 0:1], dummy[:, 0:2].bitcast(f32),
                                 Af.Relu)

            W = lambda k: wt[:, k * 128:(k + 1) * 128]
            bias = lambda k: bt[:, k:k + 1]
            wdr_pair = wdr[:, 0:256].rearrange("f (j m) -> f j m", j=2)
            wdr_one = wdr[:, 256:384]

            # ---- HAM warm-up fillers: accumulate into a dedicated PSUM
            # bank, no eviction needed. `w_ap`/`src` choose the stationary /
            # moving operands: reusing the neighbouring real matmuls'
            # stationary makes a filler cost zero LDWEIGHTS; reading freshly
            # produced data pins the filler to that point of the pipeline.
            fps = psF.tile([128, 512], f32, tag="psF")

            def pe_filler(n=1, w_ap=None, src=None):
                mv = src if src is not None else dummy[:, :256]
                st = w_ap if w_ap is not None else dummy[:, :128]
                fd = mv.shape[-1]
                for _ in range(n):
                    nc.tensor.matmul(fps[:, :fd], st, mv,
                                     start=True, stop=True,
                                     skip_group_check=True)

            pe_filler(22)  # warm-up bridging the whole input-DMA wait:
                           # PE must be at 2.4 GHz when x lands (~10us)

            t_enc = bigs.tile([128, NODES], f16, tag="t_enc")
            u_t = bigs.tile([128, NODES], f16, tag="u_t")
            v_t = bigs.tile([128, NODES], f16, tag="v_t")
            h1_t = bigs.tile([128, ECOLS], f16, tag="h1_t")
            h2_t = bigs.tile([128, ECOLS], f8, tag="h2_t")
            t_n1 = bigs.tile([128, NODES], f16, tag="t_n1")
            t_n2 = bigs.tile([128, NODES], f16, tag="t_n2")
            t_l1 = bigs.tile([128, NODES], f16, tag="t_l1")
            out_sb = bigs.tile([ACT, NODES], f32, tag="out_sb")

            def evict(eng, dst, src, bias_ap, relu, scale=None):
                if eng == "act":
                    if scale is not None:
                        nc.scalar.activation(dst, src, Af.Relu if relu else
                                             Af.Identity, bias=bias_ap,
                                             scale=scale)
                    elif relu:
                        nc.scalar.activation(dst, src, Af.Relu, bias=bias_ap)
                    elif bias_ap is not None:
                        nc.scalar.activation(dst, src, Af.Identity,
                                             bias=bias_ap)
                    else:
                        nc.scalar.copy(dst, src)
                else:
                    assert scale is None
                    if relu:
                        nc.vector.tensor_scalar(dst, src, bias_ap, 0.0,
                                                Op.add, Op.max)
                    elif bias_ap is not None:
                        nc.vector.tensor_scalar_add(dst, src, bias_ap)
                    else:
                        nc.vector.tensor_copy(dst, src)

            def node_layer(w_ap, src, dst, bias_idx, relu, engines,
                           scale=None):
                """2560-col dense layer as FD<=1024 PSUM groups."""
                for gi, g0 in enumerate(range(0, NODES, GW)):
                    gw = min(GW, NODES - g0)
                    ps = psA.tile([128, GW], f32, tag="psA")
                    for o in range(0, gw, 512):
                        nw = min(512, gw - o)
                        nc.tensor.matmul(ps[:, o:o + nw], w_ap,
                                         src[:, g0 + o:g0 + o + nw],
                                         start=True, stop=True)
                    evict(engines[gi], dst[:, g0:g0 + gw], ps[:, :gw],
                          bias(bias_idx), relu, scale=scale)

            # ---- node encoder + edge layer-1 node halves (w_in2 folded in);
            # u before v: u's eviction tail overlaps v's matmuls, and the
            # adds only need v's first group to start.
            node_layer(W(W1T), x_fm, t_enc, B1, True, EV_ENC)
            pe_filler(1, src=t_enc[:, 0:256])
            node_layer(W(WE1T), t_enc, u_t, BU, False, EV_U)
            pe_filler(1, src=u_t[:, 0:256])
            node_layer(W(WE1B), t_enc, v_t, BV, False, EV_V)
            pe_filler(1, src=v_t[:, 0:256])

            # ---- h1 = relu(u[s] + v[r]) over (r, s', p) columns, where the
            # 9 sender slots s' skip s == r (no diagonal is ever computed).
            v3 = v_t[:].rearrange("f (r p) -> f r p", p=NP_CORE)

            def tt_r(r):
                w0 = r * QB
                vb1 = v3[:, r:r + 1, :]
                ranges = [(0, r, w0), (r + 1, A, w0 + r * NP_CORE)]
                if r < 2:
                    # split the long range at the u-eviction-group boundary
                    # (s-block 8 = u col 2048) so the first piece only waits
                    # on u groups 0-1 and the adds start ~1us earlier
                    lo, hi, d0 = ranges.pop()
                    ranges += [(lo, 8, d0), (8, hi, d0 + (8 - lo) * NP_CORE)]
                first = True
                for lo, hi, d0 in ranges:
                    k = hi - lo
                    if k == 0:
                        continue
                    o = h1_t[:, d0:d0 + k * NP_CORE] \
                        .rearrange("f (s p) -> f s p", p=NP_CORE)
                    us = u_t[:, lo * NP_CORE:hi * NP_CORE] \
                        .rearrange("f (s p) -> f s p", p=NP_CORE)
                    nc.vector.tensor_add(o, us,
                                         vb1.broadcast_to([128, k, NP_CORE]))
                    if r < 2 and first:
                        pe_filler(1, w_ap=W(WE2S),
                                  src=h1_t[:, d0:d0 + 128])
                    first = False

                def relu_part(flat):
                    eng = RELU_ENG[r]
                    if eng == "act":
                        nc.scalar.activation(flat, flat, Af.Relu)
                    else:
                        nc.vector.tensor_scalar_max(flat, flat, 0.0)
                if r < 2:
                    # split so the first e2 groups unblock sooner
                    relu_part(h1_t[:, w0:w0 + GW])
                    relu_part(h1_t[:, w0 + GW:w0 + QB])
                else:
                    relu_part(h1_t[:, w0:w0 + QB])

            pe_filler(1, src=u_t[:, 2304:2560])
            pe_filler(1, src=v_t[:, 2304:2560])
            for r in range(A):
                tt_r(r)
                if r < 3:
                    pe_filler(2, src=h1_t[:, r * QB:r * QB + 256])

            # ---- main stream: h2 = relu(w_e2s^T h1 + b_e2s) stored fp8;
            # after the groups covering r-blocks {2c, 2c+1}, the fused
            # agg+n1 chunk runs (fp8 DoubleRow over sender-slot pairs), and
            # per completed t_n1 slab the rest of the network.
            h2v = h2_t[:].rearrange("f (r s p) -> f s r p", s=A - 1,
                                    p=NP_CORE)

            def agg_chunk(c):
                ps = psB.tile([128, 512], f32, tag="psB")
                nc.tensor.matmul(ps[:], W(WN1AS),
                                 t_enc[:, c * 512:(c + 1) * 512],
                                 start=True, stop=False)
                for ri, r in enumerate((2 * c, 2 * c + 1)):
                    po = ri * NP_CORE
                    for a2 in range(4):
                        nc.tensor.matmul(
                            ps[:, po:po + NP_CORE], wdr_pair,
                            h2v[:, 2 * a2:2 * a2 + 2, r:r + 1, :],
                            start=False, stop=False, perf_mode=DR)
                    nc.tensor.matmul(ps[:, po:po + NP_CORE], wdr_one,
                                     h2v[:, 8:9, r:r + 1, :],
                                     start=False, stop=(ri == 1))
                evict("act", t_n1[:, c * 512:(c + 1) * 512], ps[:],
                      bias(BN1), True, scale=S_N1)

            def slab(s0, sw, step):
                """node-MLP tail + decoder for t_n1 cols [s0, s0+sw).
                Stage-major over the slab so each weight loads once."""
                steps = list(range(s0, s0 + sw, step))
                for si, c0 in enumerate(steps):
                    ps = psB.tile([128, 512], f32, tag="psB")
                    nc.tensor.matmul(ps[:, :step], W(WN2),
                                     t_n1[:, c0:c0 + step],
                                     start=True, stop=True)
                    evict(EV_N2, t_n2[:, c0:c0 + step], ps[:, :step],
                          bias(BN2), True)
                for si, c0 in enumerate(steps):
                    ps = psB.tile([128, 512], f32, tag="psB")
                    nc.tensor.matmul(ps[:, :step], W(WL1),
                                     t_n2[:, c0:c0 + step],
                                     start=True, stop=True)
                    evict(EV_L1[(c0 // 512) % len(EV_L1)],
                          t_l1[:, c0:c0 + step], ps[:, :step],
                          bias(BL1), True)
                for si, c0 in enumerate(steps):
                    ps = psB.tile([128, 512], f32, tag="psB")
                    nc.tensor.matmul(ps[:, :step], W(WL2),
                                     t_l1[:, c0:c0 + step],
                                     start=True, stop=True)
                    if EV_OUT == "act":
                        nc.scalar.activation(out_sb[:, c0:c0 + step],
                                             ps[:ACT, :step], Af.Identity,
                                             bias=bt[0:ACT, BL2:BL2 + 1])
                    else:
                        nc.vector.tensor_scalar_add(out_sb[:, c0:c0 + step],
                                                    ps[:ACT, :step],
                                                    bt[0:ACT, BL2:BL2 + 1])
                    nc.sync.dma_start(out=out_dram[:, c0:c0 + step],
                                      in_=out_sb[:, c0:c0 + step])

            # agg+n1 chunk c needs h2 r-blocks {2c, 2c+1} = cols up to
            # (2c+2)*QB; fire it after the GW-col group covering that.
            agg_after = {((2 * c + 2) * QB - 1) // GW: c for c in range(5)}
            for g in range(N_EG):
                g0 = g * GW
                gw = min(GW, ECOLS - g0)
                ps = psA.tile([128, GW], f32, tag="psA")
                for o in range(0, gw, 512):
                    nw = min(512, gw - o)
                    nc.tensor.matmul(ps[:, o:o + nw], W(WE2S),
                                     h1_t[:, g0 + o:g0 + o + nw],
                                     start=True, stop=True)
                evict(EV_H2[g], h2_t[:, g0:g0 + gw], ps[:, :gw],
                      bias(BE2S), True)
                # zero-LDWEIGHTS filler (stationary stays WE2S), gated on
                # freshly written h1 so it lands here in the PE stream
                if g + 1 < N_EG:
                    pe_filler(1, w_ap=W(WE2S), src=h1_t[:, g0:g0 + 128])
                if g in agg_after:
                    c = agg_after[g]
                    agg_chunk(c)
                    pe_filler(1, w_ap=W(WE2S), src=t_n1[:, c * 512:c * 512 + 128])
                    if c == 1:
                        slab(0, 1024, 512)
                    elif c == 3:
                        slab(1024, 1024, 512)
                    elif c == 4:
                        slab(2048, 512, 256)

    nc.compile()
    _dedupe_ldweights(nc)
    return nc


def _dedupe_ldweights(nc):
    """Remove redundant PE weight loads after bacc splits matmuls into
    Ldweights+Matmult pairs: a Ldweights whose source AP equals the
    previously loaded one (PE stream order == block order) is a no-op.
    Only drop instructions carrying no semaphore waits/updates."""
    from concourse import mybir
    import bass_rust
    for f in nc.m.functions:
        for b in f.blocks:
            last = None
            keep = []
            insts = b.instructions
            for idx, i in enumerate(insts):
                if isinstance(i, mybir.InstLdweights):
                    key = str(i.ins[0])
                    if key == last:
                        if i.sync_info is None:
                            continue
                        # migrate waits/updates onto the paired matmult so
                        # the redundant load can still be dropped
                        nxt = insts[idx + 1] if idx + 1 < len(insts) else None
                        if isinstance(nxt, mybir.InstMatmult):
                            ow = list(i.sync_info.on_wait)
                            ou = list(i.sync_info.on_update)
                            if nxt.sync_info is not None:
                                ow += list(nxt.sync_info.on_wait)
                                ou += list(nxt.sync_info.on_update)
                            if len(ow) <= 1:    # walrus: one wait per inst
                                nxt.sync_info = bass_rust.SyncInfo(
                                    on_wait=ow, on_update=ou)
                                continue
                    last = key
                keep.append(i)
            if len(keep) != len(insts):
                b.instructions[:] = keep


def _get_program():
    global _PROG
    if _PROG is None:
        _PROG = _build_program()
    return _PROG


# ------------------------------------------------------------------- kernel

def kernel(theta, s, i, senders, receivers,
           w_in1, b_in1, w_in2, b_in2,
           w_e1, b_e1, w_e2, b_e2, w_e3, b_e3,
           w_n1, b_n1, w_n2, b_n2, w_n3, b_n3,
           w_l1, b_l1, w_l2, b_l2):
    global LAST_EXEC_NS
    import os
    import ml_dtypes

    args = dict(theta=theta, s=s, i=i, senders=senders, receivers=receivers,
                w_in1=w_in1, b_in1=b_in1, w_in2=w_in2, b_in2=b_in2,
                w_e1=w_e1, b_e1=b_e1, w_e2=w_e2, b_e2=b_e2,
                w_e3=w_e3, b_e3=b_e3, w_n1=w_n1, b_n1=b_n1,
                w_n2=w_n2, b_n2=b_n2, w_n3=w_n3, b_n3=b_n3,
                w_l1=w_l1, b_l1=b_l1, w_l2=w_l2, b_l2=b_l2)
    args = {k: np.asarray(v) for k, v in args.items()}

    # The device program hardcodes the documented block-diagonal
    # fully-connected edge structure and constant-i input; verify, else
    # fall back to a host fp32 computation (correct for any input).
    exp_s, exp_r = _expected_edges()
    i_flat = np.asarray(args["i"], np.float32).reshape(-1)
    structured = (np.array_equal(np.asarray(args["senders"], np.int64), exp_s)
                  and np.array_equal(np.asarray(args["receivers"], np.int64), exp_r)
                  and np.all(i_flat == i_flat[0]))
    if not structured:
        return _fallback_numpy(**{k: np.asarray(v, np.float32)
                                  if np.asarray(v).dtype != np.int32 else np.asarray(v)
                                  for k, v in args.items()})

    f64 = np.float64
    fp8 = ml_dtypes.float8_e4m3
    w_in1_, b_in1_ = args["w_in1"].astype(f64), args["b_in1"].astype(f64)
    w_in2_, b_in2_ = args["w_in2"].astype(f64), args["b_in2"].astype(f64)
    w_e1_, b_e1_ = args["w_e1"].astype(f64), args["b_e1"].astype(f64)
    w_e3_, b_e3_ = args["w_e3"].astype(f64), args["b_e3"].astype(f64)
    w_n1_, b_n1_ = args["w_n1"].astype(f64), args["b_n1"].astype(f64)
    w_n3_, b_n3_ = args["w_n3"].astype(f64), args["b_n3"].astype(f64)
    w_l1_, b_l1_ = args["w_l1"].astype(f64), args["b_l1"].astype(f64)

    b1_eff = b_in1_ + i_flat[0] * w_in1_[H_DIM + S_DIM]
    b_u = b_e1_ + b_in2_ @ w_e1_[:MID]
    b_v = b_in2_ @ w_e1_[MID:]
    b_n1_eff = b_n1_ + b_in2_ @ w_n1_[:MID] + b_e3_ @ w_n1_[MID:]
    b_l1_eff = b_l1_ + b_n3_ @ w_l1_

    wn1b = (w_e3_ / (A - 1)) @ w_n1_[MID:]
    wslots = np.zeros((NSLOTS, 128, 128), np.float16)
    wslots[W1T] = w_in1_[:128].astype(np.float16)
    wslots[WE1T] = (w_in2_ @ w_e1_[:MID]).astype(np.float16)
    wslots[WE1B] = (w_in2_ @ w_e1_[MID:]).astype(np.float16)
    wslots[WE2S] = (args["w_e2"].astype(f64) * S_H2).astype(np.float16)
    wslots[WN1AS] = ((w_in2_ @ w_n1_[:MID]) * (S_H2 * S_WB)).astype(np.float16)
    wslots[WN2] = args["w_n2"].astype(np.float16)
    wslots[WL1] = (w_n3_ @ w_l1_).astype(np.float16)
    wslots[WL2, :, :ACT] = args["w_l2"].astype(np.float16)
    w_pack = np.ascontiguousarray(
        wslots.transpose(1, 0, 2).reshape(128, NSLOTS * 128))

    wn1b8 = np.clip(wn1b * S_WB, -240.0, 240.0).astype(fp8)
    wdr_pack = np.ascontiguousarray(
        np.concatenate([wn1b8, wn1b8, wn1b8], axis=1))          # [128, 384]

    b_pack = np.zeros((128, 8), np.float32)
    for idx, vec in ((B1, b1_eff), (BU, b_u), (BV, b_v),
                     (BE2S, args["b_e2"].astype(f64) * S_H2),
                     (BN1, b_n1_eff), (BN2, args["b_n2"]), (BL1, b_l1_eff)):
        b_pack[:, idx] = np.asarray(vec, np.float32)
    b_pack[:ACT, BL2] = args["b_l2"].astype(np.float32)

    # node features, feat-major, (a, p) column order, per-core shards
    n_all = B * P * A
    X = np.concatenate([args["theta"].reshape(n_all, H_DIM),
                        args["s"].reshape(n_all, S_DIM)], axis=-1)
    in_maps = []
    for c in range(N_CORES):
        xc = X[c * NODES:(c + 1) * NODES]
        in_maps.append({
            "x_fm": _to_ap_major(xc).astype(np.float16),
            "w_pack": w_pack,
            "wdr_pack": wdr_pack,
            "b_pack": b_pack,
        })

    nc = _get_program()
    if os.environ.get("KERNEL_SIM", "0") == "1":
        # CoreSim core 0 only (cores are identical up to data); other cores
        # return zeros. For correctness devloop, not grading.
        from concourse import bass_interp
        sim = bass_interp.CoreSim(nc)
        for k, v in in_maps[0].items():
            sim.tensor(k)[:] = v
        sim.simulate()
        results = [{"out": np.array(sim.tensor("out"))}]
        results += [{"out": np.zeros((ACT, NODES), np.float32)}
                    for _ in range(N_CORES - 1)]
        parts = [_from_ap_major(r["out"]) for r in results]
        return np.concatenate(parts, axis=0).reshape(B, P, A, ACT).astype(np.float32)

    from concourse.bass_utils import run_bass_kernel_spmd
    trace = os.environ.get("KERNEL_TRACE", "0") == "1"
    res = run_bass_kernel_spmd(nc, in_maps, core_ids=list(range(N_CORES)),
                               trace=trace)
    LAST_EXEC_NS = res.exec_time_ns

    parts = [_from_ap_major(res.results[c]["out"]) for c in range(N_CORES)]
    return np.concatenate(parts, axis=0).reshape(B, P, A, ACT).astype(np.float32)


# revision 79
# speedup vs baseline: 1.0428x; 1.0034x over previous
"""Trainium2 Bass kernel for nn_ActionPredictionNet (GNN message passing).

Data-parallel over batch*particles: 8 NeuronCores, each handling 256
independent fully-connected 10-node particle graphs (2560 nodes, 23040
edges). The fully-connected structure lets us restructure the math:

  - Edge-MLP layer 1 collapses: e_in = [n[s], n[r]] so layer-1 pre-act is
    u[s] + v[r] with u = W_top^T n, v = W_bot^T n computed per NODE
    (2560 cols) instead of per EDGE (23040 cols), then a broadcast-add.
  - Edges are only consumed via the mean over incoming messages, so edge
    layer 3 folds into the aggregation: accumulate (sum_s h2_s) @ (w_e3/9)
    in PSUM. The aggregation matmuls run in fp8 DoubleRow mode (two sender
    slots per pass), halving their PE time; h2 is stored fp8 with a 4x
    scale folded into w_e2/b_e2 and 64x into wn1b, compensated by a 1/256
    scale on the n1 eviction.
  - Diagonal (s == r) pairs are never computed: per receiver the sender
    range splits into two dense pieces.

Layouts (per core, feat-major: features on SBUF partitions):
  - node tensors [128, 2560], column = a*256 + p  (a: node-in-graph 0..9,
    p: graph 0..255)  -> broadcast APs get innermost unit stride.
  - edge tensors [128, 23040], column = r*2304 + s'*256 + p (s' skips r).

Schedule notes (from perfetto traces of the previous version):
  - input DMA is issued from three engines in parallel (scalar / sync /
    gpsimd) so the first enc matmul can start ~5us in instead of ~11us.
  - PE HAM warm-up fillers accumulate into a dedicated PSUM bank (no
    eviction sink needed); in the e2 stream they reuse the currently
    loaded stationary weights so they cost no LDWEIGHTS.
  - PSUM evictions are the bottleneck (~1.1-1.3 ns/col on ACT/DVE, PSUM
    read port is 1 elem/cycle); they are batched at FD=1280 and routed
    across ACT/DVE by tunable tables; h1 relus run on DVE (fp16 4x mode),
    optionally a few on GPSIMD.
"""

import numpy as np

B, P, A = 32, 64, 10
S_DIM, H_DIM, MID = 64, 64, 128
ACT = 8
N_CORES = 8
NP_CORE = B * P // N_CORES          # 256 particle-graphs per core
NODES = NP_CORE * A                 # 2560 nodes per core
QB = (A - 1) * NP_CORE              # 2304 edge columns per receiver block
ECOLS = A * QB                      # 23040 (r, s', p) edge columns per core

GW = 1024                           # eviction group width (2 PSUM banks)
N_EG = (ECOLS + GW - 1) // GW       # 23 edge groups (last one 512 wide)

# fp8 scaling for the aggregation path
S_H2 = 4.0                          # h2 stored as 4*h2 (folded into w_e2/b_e2)
S_WB = 64.0                         # wn1b stored as 64*wn1b
S_N1 = 1.0 / (S_H2 * S_WB)          # eviction scale on the n1 pre-act
S_W1 = 16.0                         # w_in1 stored fp8 as 16*w_in1

_PROG = None        # cached compiled program: (nc, meta)
LAST_EXEC_NS = None  # filled when KERNEL_TRACE=1


# ------------------------------------------------------------ tuning tables
# eviction engine per group: enc(2), u(2), v(2), h2(18); relu engine per r
EV_ENC = ["act", "vec", "act"]
EV_U = ["vec", "act", "vec"]
EV_V = ["act", "vec", "vec"]
EV_H2 = ["act"] * 20 + ["act", "vec", "act"]
RELU_ENG = ["vec", "vec", "vec", "vec", "vec", "vec",
            "vec", "vec", "vec", "vec"]  # per receiver block
EV_N2 = "vec"
EV_L1 = ["vec", "vec", "vec", "vec", "vec"]   # per 512-col slab step
EV_OUT = "vec"


# ---------------------------------------------------------------- host utils

def _expected_edges():
    a = np.arange(A)
    s, r = np.meshgrid(a, a, indexing="ij")
    m = s != r
    s, r = s[m], r[m]
    offs = (np.arange(B * P) * A)[:, None]
    return (offs + s[None, :]).reshape(-1).astype(np.int64), \
           (offs + r[None, :]).reshape(-1).astype(np.int64)


def _to_ap_major(x_core):
    """[2560, D] in (p, a) node order -> [D, 2560] feat-major, (a, p) cols."""
    return np.ascontiguousarray(
        x_core.reshape(NP_CORE, A, -1).transpose(1, 0, 2).reshape(NODES, -1).T
    )


def _from_ap_major(out_core):
    """[ACT, 2560] feat-major (a, p) cols -> [2560, ACT] in (p, a) order."""
    return out_core.T.reshape(A, NP_CORE, ACT).transpose(1, 0, 2).reshape(NODES, ACT)


def _fallback_numpy(theta, s, i, senders, receivers,
                    w_in1, b_in1, w_in2, b_in2,
                    w_e1, b_e1, w_e2, b_e2, w_e3, b_e3,
                    w_n1, b_n1, w_n2, b_n2, w_n3, b_n3,
                    w_l1, b_l1, w_l2, b_l2):
    """fp32 numpy replica of the reference; used only if inputs deviate from
    the documented structure (non-fully-connected edges or non-constant i)."""
    N = B * P * A
    relu = lambda x: np.maximum(x, 0.0)
    x = np.concatenate([theta.reshape(N, H_DIM), s.reshape(N, S_DIM),
                        i.reshape(N, 1)], axis=-1).astype(np.float32)
    n = relu(x @ w_in1 + b_in1) @ w_in2 + b_in2
    e_in = np.concatenate([n[senders], n[receivers]], axis=-1)
    e = relu(e_in @ w_e1 + b_e1)
    e = relu(e @ w_e2 + b_e2)
    e = e @ w_e3 + b_e3
    agg = np.zeros((N, e.shape[1]), np.float32)
    np.add.at(agg, receivers, e)
    agg /= (A - 1)
    h = np.concatenate([n, agg], axis=-1)
    h = relu(h @ w_n1 + b_n1)
    h = relu(h @ w_n2 + b_n2)
    h = h @ w_n3 + b_n3
    out = relu(h @ w_l1 + b_l1) @ w_l2 + b_l2
    return out.reshape(B, P, A, ACT).astype(np.float32)


# ------------------------------------------------------------- device program

# fp16 weight-pack slot indices (linear-linear layer pairs folded on host:
# w_in2 into we1t/we1b/wn1a, w_n3 into w_l1; w_e3/9 lives in the fp8 pack)
W1T, WE1T, WE1B, WE2S, WN1AS, WN2, WL1, WL2 = range(8)
NSLOTS = 8
# bias-pack column indices
B1, BU, BV, BE2S, BN1, BN2, BL1, BL2 = range(8)


def _build_program():
    import concourse.bass as bass
    import concourse.mybir as mybir
    import concourse.tile as tile
    from concourse import bacc

    f16 = mybir.dt.float16
    f32 = mybir.dt.float32
    f8 = mybir.dt.float8e4
    Af = mybir.ActivationFunctionType
    Op = mybir.AluOpType
    DR = mybir.MatmulPerfMode.DoubleRow

    nc = bacc.Bacc("TRN2", target_bir_lowering=False, debug=False)
    x_dram = nc.dram_tensor("x_fm", [128, NODES], f16, kind="ExternalInput").ap()
    w_dram = nc.dram_tensor("w_pack", [128, NSLOTS * 128], f16,
                            kind="ExternalInput").ap()
    wdr_dram = nc.dram_tensor("wdr_pack", [128, 384], f8,
                              kind="ExternalInput").ap()
    b_dram = nc.dram_tensor("b_pack", [128, 8], f32, kind="ExternalInput").ap()
    out_dram = nc.dram_tensor("out", [ACT, NODES], f32, kind="ExternalOutput").ap()

    with tile.TileContext(nc) as tc:
        with (
            tc.tile_pool(name="consts", bufs=1) as consts,
            tc.tile_pool(name="bigs", bufs=1) as bigs,
            tc.tile_pool(name="psA", bufs=2, space="PSUM") as psA,
            tc.tile_pool(name="psB", bufs=3, space="PSUM") as psB,
            tc.tile_pool(name="psF", bufs=1, space="PSUM") as psF,
        ):
            wt = consts.tile([128, NSLOTS * 128], f16, tag="wt")
            wdr = consts.tile([128, 384], f8, tag="wdr")
            bt = consts.tile([128, 8], f32, tag="bt")
            x_fm = bigs.tile([128, NODES], f16, tag="x_fm")
            dummy = consts.tile([128, 256], f16, tag="dummy")
            dsink = consts.tile([128, 8], f32, tag="dsink")

            # ---- input DMA, spread across three issuing engines so the
            # transfers overlap; first enc group only needs x[:, :1280].
            nc.gpsimd.memset(dummy[:], 0.0)
            nc.gpsimd.dma_start(out=x_fm[:, 1920:2560], in_=x_dram[:, 1920:2560])
            nc.scalar.dma_start(out=x_fm[:, 0:1024], in_=x_dram[:, 0:1024])
            nc.scalar.dma_start(out=bt[:], in_=b_dram)
            nc.sync.dma_start(out=wt[:, :3 * 128], in_=w_dram[:, :3 * 128])
            nc.sync.dma_start(out=x_fm[:, 1024:1920], in_=x_dram[:, 1024:1920])
            nc.sync.dma_start(out=wt[:, 3 * 128:], in_=w_dram[:, 3 * 128:])
            nc.sync.dma_start(out=wdr[:], in_=wdr_dram)

            # touch the Relu table set early so ACT_TABLE_LOAD hides in the
            # DMA-wait head instead of stalling the first real eviction
            nc.scalar.activation(dsink[:, 0:1], dummy[:, 0:2].bitcast(f32),
                                 Af.Relu)

            W = lambda k: wt[:, k * 128:(k + 1) * 128]
            bias = lambda k: bt[:, k:k + 1]
            wdr_pair = wdr[:, 0:256].rearrange("f (j m) -> f j m", j=2)
            wdr_one = wdr[:, 256:384]

            # ---- HAM warm-up fillers: accumulate into a dedicated PSUM
            # bank, no eviction needed. `w_ap`/`src` choose the stationary /
            # moving operands: reusing the neighbouring real matmuls'
            # stationary makes a filler cost zero LDWEIGHTS; reading freshly
            # produced data pins the filler to that point of the pipeline.
            fps = psF.tile([128, 512], f32, tag="psF")

            def pe_filler(n=1, w_ap=None, src=None):
                mv = src if src is not None else dummy[:, :256]
                st = w_ap if w_ap is not None else dummy[:, :128]
                fd = mv.shape[-1]
                for _ in range(n):
                    nc.tensor.matmul(fps[:, :fd], st, mv,
                                     start=True, stop=True,
                                     skip_group_check=True)

            pe_filler(22)  # warm-up bridging the whole input-DMA wait:
                           # PE must be at 2.4 GHz when x lands (~10us)

            t_enc = bigs.tile([128, NODES], f16, tag="t_enc")
            u_t = bigs.tile([128, NODES], f16, tag="u_t")
            v_t = bigs.tile([128, NODES], f16, tag="v_t")
            h1_t = bigs.tile([128, ECOLS], f16, tag="h1_t")
            h2_t = bigs.tile([128, ECOLS], f8, tag="h2_t")
            t_n1 = bigs.tile([128, NODES], f16, tag="t_n1")
            t_n2 = bigs.tile([128, NODES], f16, tag="t_n2")
            t_l1 = bigs.tile([128, NODES], f16, tag="t_l1")
            out_sb = bigs.tile([ACT, NODES], f32, tag="out_sb")

            def evict(eng, dst, src, bias_ap, relu, scale=None):
                if eng == "act":
                    if scale is not None:
                        nc.scalar.activation(dst, src, Af.Relu if relu else
                                             Af.Identity, bias=bias_ap,
                                             scale=scale)
                    elif relu:
                        nc.scalar.activation(dst, src, Af.Relu, bias=bias_ap)
                    elif bias_ap is not None:
                        nc.scalar.activation(dst, src, Af.Identity,
                                             bias=bias_ap)
                    else:
                        nc.scalar.copy(dst, src)
                else:
                    assert scale is None
                    if relu:
                        nc.vector.tensor_scalar(dst, src, bias_ap, 0.0,
                                                Op.add, Op.max)
                    elif bias_ap is not None:
                        nc.vector.tensor_scalar_add(dst, src, bias_ap)
                    else:
                        nc.vector.tensor_copy(dst, src)

            def node_layer(w_ap, src, dst, bias_idx, relu, engines,
                           scale=None):
                """2560-col dense layer as FD<=1024 PSUM groups."""
                for gi, g0 in enumerate(range(0, NODES, GW)):
                    gw = min(GW, NODES - g0)
                    ps = psA.tile([128, GW], f32, tag="psA")
                    for o in range(0, gw, 512):
                        nw = min(512, gw - o)
                        nc.tensor.matmul(ps[:, o:o + nw], w_ap,
                                         src[:, g0 + o:g0 + o + nw],
                                         start=True, stop=True)
                    evict(engines[gi], dst[:, g0:g0 + gw], ps[:, :gw],
                          bias(bias_idx), relu, scale=scale)

            # ---- node encoder + edge layer-1 node halves (w_in2 folded in);
            # u before v: u's eviction tail overlaps v's matmuls, and the
            # adds only need v's first group to start.
            node_layer(W(W1T), x_fm, t_enc, B1, True, EV_ENC)
            pe_filler(1, src=t_enc[:, 0:256])
            node_layer(W(WE1T), t_enc, u_t, BU, False, EV_U)
            pe_filler(1, src=u_t[:, 0:256])
            node_layer(W(WE1B), t_enc, v_t, BV, False, EV_V)
            pe_filler(1, src=v_t[:, 0:256])

            # ---- h1 = relu(u[s] + v[r]) over (r, s', p) columns, where the
            # 9 sender slots s' skip s == r (no diagonal is ever computed).
            v3 = v_t[:].rearrange("f (r p) -> f r p", p=NP_CORE)

            def tt_r(r):
                w0 = r * QB
                vb1 = v3[:, r:r + 1, :]
                ranges = [(0, r, w0), (r + 1, A, w0 + r * NP_CORE)]
                if r < 2:
                    # split the long range at the u-eviction-group boundary
                    # (s-block 8 = u col 2048) so the first piece only waits
                    # on u groups 0-1 and the adds start ~1us earlier
                    lo, hi, d0 = ranges.pop()
                    ranges += [(lo, 8, d0), (8, hi, d0 + (8 - lo) * NP_CORE)]
                def relu_part(flat):
                    eng = RELU_ENG[r]
                    if eng == "act":
                        nc.scalar.activation(flat, flat, Af.Relu)
                    else:
                        nc.vector.tensor_scalar_max(flat, flat, 0.0)

                first = True
                part_a = False
                for lo, hi, d0 in ranges:
                    k = hi - lo
                    if k == 0:
                        continue
                    o = h1_t[:, d0:d0 + k * NP_CORE] \
                        .rearrange("f (s p) -> f s p", p=NP_CORE)
                    us = u_t[:, lo * NP_CORE:hi * NP_CORE] \
                        .rearrange("f (s p) -> f s p", p=NP_CORE)
                    nc.vector.tensor_add(o, us,
                                         vb1.broadcast_to([128, k, NP_CORE]))
                    if r < 2 and first:
                        pe_filler(1, w_ap=W(WE2S),
                                  src=h1_t[:, d0:d0 + 128])
                    first = False
                    # interleave the first relu piece into the add stream so
                    # the first e2 groups unblock before the block finishes
                    if r < 2 and not part_a and \
                            d0 + k * NP_CORE >= w0 + GW:
                        relu_part(h1_t[:, w0:w0 + GW])
                        part_a = True
                if part_a:
                    relu_part(h1_t[:, w0 + GW:w0 + QB])
                else:
                    relu_part(h1_t[:, w0:w0 + QB])

            pe_filler(1, src=u_t[:, 2304:2560])
            pe_filler(1, src=v_t[:, 2304:2560])
            for r in range(A):
                tt_r(r)
                if r < 3:
                    pe_filler(2, src=h1_t[:, r * QB:r * QB + 256])

            # ---- main stream: h2 = relu(w_e2s^T h1 + b_e2s) stored fp8;
            # after the groups covering r-blocks {2c, 2c+1}, the fused
            # agg+n1 chunk runs (fp8 DoubleRow over sender-slot pairs), and
            # per completed t_n1 slab the rest of the network.
            h2v = h2_t[:].rearrange("f (r s p) -> f s r p", s=A - 1,
                                    p=NP_CORE)

            def agg_chunk(c):
                ps = psB.tile([128, 512], f32, tag="psB")
                nc.tensor.matmul(ps[:], W(WN1AS),
                                 t_enc[:, c * 512:(c + 1) * 512],
                                 start=True, stop=False)
                for ri, r in enumerate((2 * c, 2 * c + 1)):
                    po = ri * NP_CORE
                    for a2 in range(4):
                        nc.tensor.matmul(
                            ps[:, po:po + NP_CORE], wdr_pair,
                            h2v[:, 2 * a2:2 * a2 + 2, r:r + 1, :],
                            start=False, stop=False, perf_mode=DR)
                    nc.tensor.matmul(ps[:, po:po + NP_CORE], wdr_one,
                                     h2v[:, 8:9, r:r + 1, :],
                                     start=False, stop=(ri == 1))
                evict("act", t_n1[:, c * 512:(c + 1) * 512], ps[:],
                      bias(BN1), True, scale=S_N1)

            def slab(s0, sw, step):
                """node-MLP tail + decoder for t_n1 cols [s0, s0+sw).
                Stage-major over the slab so each weight loads once."""
                steps = list(range(s0, s0 + sw, step))
                for si, c0 in enumerate(steps):
                    ps = psB.tile([128, 512], f32, tag="psB")
                    nc.tensor.matmul(ps[:, :step], W(WN2),
                                     t_n1[:, c0:c0 + step],
                                     start=True, stop=True)
                    evict(EV_N2, t_n2[:, c0:c0 + step], ps[:, :step],
                          bias(BN2), True)
                for si, c0 in enumerate(steps):
                    ps = psB.tile([128, 512], f32, tag="psB")
                    nc.tensor.matmul(ps[:, :step], W(WL1),
                                     t_n2[:, c0:c0 + step],
                                     start=True, stop=True)
                    evict(EV_L1[(c0 // 512) % len(EV_L1)],
                          t_l1[:, c0:c0 + step], ps[:, :step],
                          bias(BL1), True)
                for si, c0 in enumerate(steps):
                    ps = psB.tile([128, 512], f32, tag="psB")
                    nc.tensor.matmul(ps[:, :step], W(WL2),
                                     t_l1[:, c0:c0 + step],
                                     start=True, stop=True)
                    if EV_OUT == "act":
                        nc.scalar.activation(out_sb[:, c0:c0 + step],
                                             ps[:ACT, :step], Af.Identity,
                                             bias=bt[0:ACT, BL2:BL2 + 1])
                    else:
                        nc.vector.tensor_scalar_add(out_sb[:, c0:c0 + step],
                                                    ps[:ACT, :step],
                                                    bt[0:ACT, BL2:BL2 + 1])
                    nc.sync.dma_start(out=out_dram[:, c0:c0 + step],
                                      in_=out_sb[:, c0:c0 + step])

            # agg+n1 chunk c needs h2 r-blocks {2c, 2c+1} = cols up to
            # (2c+2)*QB; fire it after the GW-col group covering that.
            agg_after = {((2 * c + 2) * QB - 1) // GW: c for c in range(5)}
            for g in range(N_EG):
                g0 = g * GW
                gw = min(GW, ECOLS - g0)
                ps = psA.tile([128, GW], f32, tag="psA")
                for o in range(0, gw, 512):
                    nw = min(512, gw - o)
                    nc.tensor.matmul(ps[:, o:o + nw], W(WE2S),
                                     h1_t[:, g0 + o:g0 + o + nw],
                                     start=True, stop=True)
                evict(EV_H2[g], h2_t[:, g0:g0 + gw], ps[:, :gw],
                      bias(BE2S), True)
                # zero-LDWEIGHTS filler (stationary stays WE2S), gated on
                # freshly written h1 so it lands here in the PE stream
                if g + 1 < N_EG:
                    pe_filler(1, w_ap=W(WE2S), src=h1_t[:, g0:g0 + 128])
                if g in agg_after:
                    c = agg_after[g]
                    agg_chunk(c)
                    pe_filler(1, w_ap=W(WE2S), src=t_n1[:, c * 512:c * 512 + 128])
                    if c == 1:
                        slab(0, 1024, 512)
                    elif c == 3:
                        slab(1024, 1024, 512)
                    elif c == 4:
                        slab(2048, 512, 256)

    nc.compile()
    _dedupe_ldweights(nc)
    return nc


def _dedupe_ldweights(nc):
    """Remove redundant PE weight loads after bacc splits matmuls into
    Ldweights+Matmult pairs: a Ldweights whose source AP equals the
    previously loaded one (PE stream order == block order) is a no-op.
    Only drop instructions carrying no semaphore waits/updates."""
    from concourse import mybir
    import bass_rust
    for f in nc.m.functions:
        for b in f.blocks:
            last = None
            keep = []
            insts = b.instructions
            for idx, i in enumerate(insts):
                if isinstance(i, mybir.InstLdweights):
                    key = str(i.ins[0])
                    if key == last:
                        if i.sync_info is None:
                            continue
                        # migrate waits/updates onto the paired matmult so
                        # the redundant load can still be dropped
                        nxt = insts[idx + 1] if idx + 1 < len(insts) else None
                        if isinstance(nxt, mybir.InstMatmult):
                            ow = list(i.sync_info.on_wait)
                            ou = list(i.sync_info.on_update)
                            if nxt.sync_info is not None:
                                ow += list(nxt.sync_info.on_wait)
                                ou += list(nxt.sync_info.on_update)
                            if len(ow) <= 1:    # walrus: one wait per inst
                                nxt.sync_info = bass_rust.SyncInfo(
                                    on_wait=ow, on_update=ou)
                                continue
                    last = key
                keep.append(i)
            if len(keep) != len(insts):
                b.instructions[:] = keep


def _get_program():
    global _PROG
    if _PROG is None:
        _PROG = _build_program()
    return _PROG


# ------------------------------------------------------------------- kernel

def kernel(theta, s, i, senders, receivers,
           w_in1, b_in1, w_in2, b_in2,
           w_e1, b_e1, w_e2, b_e2, w_e3, b_e3,
           w_n1, b_n1, w_n2, b_n2, w_n3, b_n3,
           w_l1, b_l1, w_l2, b_l2):
    global LAST_EXEC_NS
    import os
    import ml_dtypes

    args = dict(theta=theta, s=s, i=i, senders=senders, receivers=receivers,
                w_in1=w_in1, b_in1=b_in1, w_in2=w_in2, b_in2=b_in2,
                w_e1=w_e1, b_e1=b_e1, w_e2=w_e2, b_e2=b_e2,
                w_e3=w_e3, b_e3=b_e3, w_n1=w_n1, b_n1=b_n1,
                w_n2=w_n2, b_n2=b_n2, w_n3=w_n3, b_n3=b_n3,
                w_l1=w_l1, b_l1=b_l1, w_l2=w_l2, b_l2=b_l2)
    args = {k: np.asarray(v) for k, v in args.items()}

    # The device program hardcodes the documented block-diagonal
    # fully-connected edge structure and constant-i input; verify, else
    # fall back to a host fp32 computation (correct for any input).
    exp_s, exp_r = _expected_edges()
    i_flat = np.asarray(args["i"], np.float32).reshape(-1)
    structured = (np.array_equal(np.asarray(args["senders"], np.int64), exp_s)
                  and np.array_equal(np.asarray(args["receivers"], np.int64), exp_r)
                  and np.all(i_flat == i_flat[0]))
    if not structured:
        return _fallback_numpy(**{k: np.asarray(v, np.float32)
                                  if np.asarray(v).dtype != np.int32 else np.asarray(v)
                                  for k, v in args.items()})

    f64 = np.float64
    fp8 = ml_dtypes.float8_e4m3
    w_in1_, b_in1_ = args["w_in1"].astype(f64), args["b_in1"].astype(f64)
    w_in2_, b_in2_ = args["w_in2"].astype(f64), args["b_in2"].astype(f64)
    w_e1_, b_e1_ = args["w_e1"].astype(f64), args["b_e1"].astype(f64)
    w_e3_, b_e3_ = args["w_e3"].astype(f64), args["b_e3"].astype(f64)
    w_n1_, b_n1_ = args["w_n1"].astype(f64), args["b_n1"].astype(f64)
    w_n3_, b_n3_ = args["w_n3"].astype(f64), args["b_n3"].astype(f64)
    w_l1_, b_l1_ = args["w_l1"].astype(f64), args["b_l1"].astype(f64)

    b1_eff = b_in1_ + i_flat[0] * w_in1_[H_DIM + S_DIM]
    b_u = b_e1_ + b_in2_ @ w_e1_[:MID]
    b_v = b_in2_ @ w_e1_[MID:]
    b_n1_eff = b_n1_ + b_in2_ @ w_n1_[:MID] + b_e3_ @ w_n1_[MID:]
    b_l1_eff = b_l1_ + b_n3_ @ w_l1_

    wn1b = (w_e3_ / (A - 1)) @ w_n1_[MID:]
    wslots = np.zeros((NSLOTS, 128, 128), np.float16)
    wslots[W1T] = w_in1_[:128].astype(np.float16)
    wslots[WE1T] = (w_in2_ @ w_e1_[:MID]).astype(np.float16)
    wslots[WE1B] = (w_in2_ @ w_e1_[MID:]).astype(np.float16)
    wslots[WE2S] = (args["w_e2"].astype(f64) * S_H2).astype(np.float16)
    wslots[WN1AS] = ((w_in2_ @ w_n1_[:MID]) * (S_H2 * S_WB)).astype(np.float16)
    wslots[WN2] = args["w_n2"].astype(np.float16)
    wslots[WL1] = (w_n3_ @ w_l1_).astype(np.float16)
    wslots[WL2, :, :ACT] = args["w_l2"].astype(np.float16)
    w_pack = np.ascontiguousarray(
        wslots.transpose(1, 0, 2).reshape(128, NSLOTS * 128))

    wn1b8 = np.clip(wn1b * S_WB, -240.0, 240.0).astype(fp8)
    wdr_pack = np.ascontiguousarray(
        np.concatenate([wn1b8, wn1b8, wn1b8], axis=1))          # [128, 384]

    b_pack = np.zeros((128, 8), np.float32)
    for idx, vec in ((B1, b1_eff), (BU, b_u), (BV, b_v),
                     (BE2S, args["b_e2"].astype(f64) * S_H2),
                     (BN1, b_n1_eff), (BN2, args["b_n2"]), (BL1, b_l1_eff)):
        b_pack[:, idx] = np.asarray(vec, np.float32)
    b_pack[:ACT, BL2] = args["b_l2"].astype(np.float32)

    # node features, feat-major, (a, p) column order, per-core shards
    n_all = B * P * A
    X = np.concatenate([args["theta"].reshape(n_all, H_DIM),
                        args["s"].reshape(n_all, S_DIM)], axis=-1)
    in_maps = []
    for c in range(N_CORES):
        xc = X[c * NODES:(c + 1) * NODES]
        in_maps.append({
            "x_fm": _to_ap_major(xc).astype(np.float16),
            "w_pack": w_pack,
            "wdr_pack": wdr_pack,
            "b_pack": b_pack,
        })

    nc = _get_program()
    if os.environ.get("KERNEL_SIM", "0") == "1":
        # CoreSim core 0 only (cores are identical up to data); other cores
        # return zeros. For correctness devloop, not grading.
        from concourse import bass_interp
        sim = bass_interp.CoreSim(nc)
        for k, v in in_maps[0].items():
            sim.tensor(k)[:] = v
        sim.simulate()
        results = [{"out": np.array(sim.tensor("out"))}]
        results += [{"out": np.zeros((ACT, NODES), np.float32)}
                    for _ in range(N_CORES - 1)]
        parts = [_from_ap_major(r["out"]) for r in results]
        return np.concatenate(parts, axis=0).reshape(B, P, A, ACT).astype(np.float32)

    from concourse.bass_utils import run_bass_kernel_spmd
    trace = os.environ.get("KERNEL_TRACE", "0") == "1"
    res = run_bass_kernel_spmd(nc, in_maps, core_ids=list(range(N_CORES)),
                               trace=trace)
    LAST_EXEC_NS = res.exec_time_ns

    parts = [_from_ap_major(res.results[c]["out"]) for c in range(N_CORES)]
    return np.concatenate(parts, axis=0).reshape(B, P, A, ACT).astype(np.float32)


# revision 80
# speedup vs baseline: 1.0666x; 1.0229x over previous
"""Trainium2 Bass kernel for nn_ActionPredictionNet (GNN message passing).

Data-parallel over batch*particles: 8 NeuronCores, each handling 256
independent fully-connected 10-node particle graphs (2560 nodes, 23040
edges). The fully-connected structure lets us restructure the math:

  - Edge-MLP layer 1 collapses: e_in = [n[s], n[r]] so layer-1 pre-act is
    u[s] + v[r] with u = W_top^T n, v = W_bot^T n computed per NODE
    (2560 cols) instead of per EDGE (23040 cols), then a broadcast-add.
  - Edges are only consumed via the mean over incoming messages, so edge
    layer 3 folds into the aggregation: accumulate (sum_s h2_s) @ (w_e3/9)
    in PSUM. The aggregation matmuls run in fp8 DoubleRow mode (two sender
    slots per pass), halving their PE time; h2 is stored fp8 with a 4x
    scale folded into w_e2/b_e2 and 64x into wn1b, compensated by a 1/256
    scale on the n1 eviction.
  - Diagonal (s == r) pairs are never computed: per receiver the sender
    range splits into two dense pieces.

Layouts (per core, feat-major: features on SBUF partitions):
  - node tensors [128, 2560], column = a*256 + p  (a: node-in-graph 0..9,
    p: graph 0..255)  -> broadcast APs get innermost unit stride.
  - edge tensors [128, 23040], column = r*2304 + s'*256 + p (s' skips r).

Schedule notes (from perfetto traces of the previous version):
  - input DMA is issued from three engines in parallel (scalar / sync /
    gpsimd) so the first enc matmul can start ~5us in instead of ~11us.
  - PE HAM warm-up fillers accumulate into a dedicated PSUM bank (no
    eviction sink needed); in the e2 stream they reuse the currently
    loaded stationary weights so they cost no LDWEIGHTS.
  - PSUM evictions are the bottleneck (~1.1-1.3 ns/col on ACT/DVE, PSUM
    read port is 1 elem/cycle); they are batched at FD=1280 and routed
    across ACT/DVE by tunable tables; h1 relus run on DVE (fp16 4x mode),
    optionally a few on GPSIMD.
"""

import numpy as np

B, P, A = 32, 64, 10
S_DIM, H_DIM, MID = 64, 64, 128
ACT = 8
N_CORES = 8
NP_CORE = B * P // N_CORES          # 256 particle-graphs per core
NODES = NP_CORE * A                 # 2560 nodes per core
QB = (A - 1) * NP_CORE              # 2304 edge columns per receiver block
ECOLS = A * QB                      # 23040 (r, s', p) edge columns per core

GW = 1024                           # eviction group width (2 PSUM banks)
N_EG = (ECOLS + GW - 1) // GW       # 23 edge groups (last one 512 wide)

# fp8 scaling for the aggregation path
S_H2 = 4.0                          # h2 stored as 4*h2 (folded into w_e2/b_e2)
S_WB = 64.0                         # wn1b stored as 64*wn1b
S_N1 = 1.0 / (S_H2 * S_WB)          # eviction scale on the n1 pre-act
S_W1 = 16.0                         # w_in1 stored fp8 as 16*w_in1

_PROG = None        # cached compiled program: (nc, meta)
LAST_EXEC_NS = None  # filled when KERNEL_TRACE=1


# ------------------------------------------------------------ tuning tables
# eviction engine per group: enc(2), u(2), v(2), h2(18); relu engine per r
EV_ENC = ["act", "vec", "act"]
EV_U = ["vec", "act", "vec"]
EV_V = ["act", "vec", "vec"]
EV_H2 = ["act"] * 20 + ["act", "vec", "act"]
RELU_ENG = ["act", "vec", "vec", "vec", "vec", "vec",
            "vec", "vec", "vec", "vec"]  # per receiver block
EV_N2 = "vec"
EV_L1 = ["vec", "vec", "vec", "vec", "vec"]   # per 512-col slab step
EV_OUT = "vec"


# ---------------------------------------------------------------- host utils

def _expected_edges():
    a = np.arange(A)
    s, r = np.meshgrid(a, a, indexing="ij")
    m = s != r
    s, r = s[m], r[m]
    offs = (np.arange(B * P) * A)[:, None]
    return (offs + s[None, :]).reshape(-1).astype(np.int64), \
           (offs + r[None, :]).reshape(-1).astype(np.int64)


def _to_ap_major(x_core):
    """[2560, D] in (p, a) node order -> [D, 2560] feat-major, (a, p) cols."""
    return np.ascontiguousarray(
        x_core.reshape(NP_CORE, A, -1).transpose(1, 0, 2).reshape(NODES, -1).T
    )


def _from_ap_major(out_core):
    """[ACT, 2560] feat-major (a, p) cols -> [2560, ACT] in (p, a) order."""
    return out_core.T.reshape(A, NP_CORE, ACT).transpose(1, 0, 2).reshape(NODES, ACT)


def _fallback_numpy(theta, s, i, senders, receivers,
                    w_in1, b_in1, w_in2, b_in2,
                    w_e1, b_e1, w_e2, b_e2, w_e3, b_e3,
                    w_n1, b_n1, w_n2, b_n2, w_n3, b_n3,
                    w_l1, b_l1, w_l2, b_l2):
    """fp32 numpy replica of the reference; used only if inputs deviate from
    the documented structure (non-fully-connected edges or non-constant i)."""
    N = B * P * A
    relu = lambda x: np.maximum(x, 0.0)
    x = np.concatenate([theta.reshape(N, H_DIM), s.reshape(N, S_DIM),
                        i.reshape(N, 1)], axis=-1).astype(np.float32)
    n = relu(x @ w_in1 + b_in1) @ w_in2 + b_in2
    e_in = np.concatenate([n[senders], n[receivers]], axis=-1)
    e = relu(e_in @ w_e1 + b_e1)
    e = relu(e @ w_e2 + b_e2)
    e = e @ w_e3 + b_e3
    agg = np.zeros((N, e.shape[1]), np.float32)
    np.add.at(agg, receivers, e)
    agg /= (A - 1)
    h = np.concatenate([n, agg], axis=-1)
    h = relu(h @ w_n1 + b_n1)
    h = relu(h @ w_n2 + b_n2)
    h = h @ w_n3 + b_n3
    out = relu(h @ w_l1 + b_l1) @ w_l2 + b_l2
    return out.reshape(B, P, A, ACT).astype(np.float32)


# ------------------------------------------------------------- device program

# fp16 weight-pack slot indices (linear-linear layer pairs folded on host:
# w_in2 into we1t/we1b/wn1a, w_n3 into w_l1; w_e3/9 lives in the fp8 pack)
W1T, WE1T, WE1B, WE2S, WN1AS, WN2, WL1, WL2 = range(8)
NSLOTS = 8
# bias-pack column indices
B1, BU, BV, BE2S, BN1, BN2, BL1, BL2 = range(8)


def _build_program():
    import concourse.bass as bass
    import concourse.mybir as mybir
    import concourse.tile as tile
    from concourse import bacc

    f16 = mybir.dt.float16
    f32 = mybir.dt.float32
    f8 = mybir.dt.float8e4
    Af = mybir.ActivationFunctionType
    Op = mybir.AluOpType
    DR = mybir.MatmulPerfMode.DoubleRow

    nc = bacc.Bacc("TRN2", target_bir_lowering=False, debug=False)
    x_dram = nc.dram_tensor("x_fm", [128, NODES], f16, kind="ExternalInput").ap()
    w_dram = nc.dram_tensor("w_pack", [128, NSLOTS * 128], f16,
                            kind="ExternalInput").ap()
    wdr_dram = nc.dram_tensor("wdr_pack", [128, 384], f8,
                              kind="ExternalInput").ap()
    b_dram = nc.dram_tensor("b_pack", [128, 8], f32, kind="ExternalInput").ap()
    out_dram = nc.dram_tensor("out", [ACT, NODES], f32, kind="ExternalOutput").ap()

    with tile.TileContext(nc) as tc:
        with (
            tc.tile_pool(name="consts", bufs=1) as consts,
            tc.tile_pool(name="bigs", bufs=1) as bigs,
            tc.tile_pool(name="psA", bufs=2, space="PSUM") as psA,
            tc.tile_pool(name="psB", bufs=3, space="PSUM") as psB,
            tc.tile_pool(name="psF", bufs=1, space="PSUM") as psF,
        ):
            wt = consts.tile([128, NSLOTS * 128], f16, tag="wt")
            wdr = consts.tile([128, 384], f8, tag="wdr")
            bt = consts.tile([128, 8], f32, tag="bt")
            x_fm = bigs.tile([128, NODES], f16, tag="x_fm")
            dummy = consts.tile([128, 256], f16, tag="dummy")
            dsink = consts.tile([128, 8], f32, tag="dsink")

            # ---- input DMA, spread across three issuing engines so the
            # transfers overlap; first enc group only needs x[:, :1280].
            nc.gpsimd.memset(dummy[:], 0.0)
            nc.gpsimd.dma_start(out=x_fm[:, 1920:2560], in_=x_dram[:, 1920:2560])
            nc.scalar.dma_start(out=x_fm[:, 0:1280], in_=x_dram[:, 0:1280])
            nc.scalar.dma_start(out=bt[:], in_=b_dram)
            nc.sync.dma_start(out=wt[:, :3 * 128], in_=w_dram[:, :3 * 128])
            nc.sync.dma_start(out=x_fm[:, 1280:1920], in_=x_dram[:, 1280:1920])
            nc.sync.dma_start(out=wt[:, 3 * 128:], in_=w_dram[:, 3 * 128:])
            nc.sync.dma_start(out=wdr[:], in_=wdr_dram)

            # touch the Relu table set early so ACT_TABLE_LOAD hides in the
            # DMA-wait head instead of stalling the first real eviction
            nc.scalar.activation(dsink[:, 0:1], dummy[:, 0:2].bitcast(f32),
                                 Af.Relu)

            W = lambda k: wt[:, k * 128:(k + 1) * 128]
            bias = lambda k: bt[:, k:k + 1]
            wdr_pair = wdr[:, 0:256].rearrange("f (j m) -> f j m", j=2)
            wdr_one = wdr[:, 256:384]

            # ---- HAM warm-up fillers: accumulate into a dedicated PSUM
            # bank, no eviction needed. `w_ap`/`src` choose the stationary /
            # moving operands: reusing the neighbouring real matmuls'
            # stationary makes a filler cost zero LDWEIGHTS; reading freshly
            # produced data pins the filler to that point of the pipeline.
            fps = psF.tile([128, 512], f32, tag="psF")

            def pe_filler(n=1, w_ap=None, src=None):
                mv = src if src is not None else dummy[:, :256]
                st = w_ap if w_ap is not None else dummy[:, :128]
                fd = mv.shape[-1]
                for _ in range(n):
                    nc.tensor.matmul(fps[:, :fd], st, mv,
                                     start=True, stop=True,
                                     skip_group_check=True)

            pe_filler(22)  # warm-up bridging the whole input-DMA wait:
                           # PE must be at 2.4 GHz when x lands (~10us)

            t_enc = bigs.tile([128, NODES], f16, tag="t_enc")
            u_t = bigs.tile([128, NODES], f16, tag="u_t")
            v_t = bigs.tile([128, NODES], f16, tag="v_t")
            h1_t = bigs.tile([128, ECOLS], f16, tag="h1_t")
            h2_t = bigs.tile([128, ECOLS], f8, tag="h2_t")
            t_n1 = bigs.tile([128, NODES], f16, tag="t_n1")
            t_n2 = bigs.tile([128, NODES], f16, tag="t_n2")
            t_l1 = bigs.tile([128, NODES], f16, tag="t_l1")
            out_sb = bigs.tile([ACT, NODES], f32, tag="out_sb")

            def evict(eng, dst, src, bias_ap, relu, scale=None):
                if eng == "act":
                    if scale is not None:
                        nc.scalar.activation(dst, src, Af.Relu if relu else
                                             Af.Identity, bias=bias_ap,
                                             scale=scale)
                    elif relu:
                        nc.scalar.activation(dst, src, Af.Relu, bias=bias_ap)
                    elif bias_ap is not None:
                        nc.scalar.activation(dst, src, Af.Identity,
                                             bias=bias_ap)
                    else:
                        nc.scalar.copy(dst, src)
                else:
                    assert scale is None
                    if relu:
                        nc.vector.tensor_scalar(dst, src, bias_ap, 0.0,
                                                Op.add, Op.max)
                    elif bias_ap is not None:
                        nc.vector.tensor_scalar_add(dst, src, bias_ap)
                    else:
                        nc.vector.tensor_copy(dst, src)

            def node_layer(w_ap, src, dst, bias_idx, relu, engines,
                           scale=None):
                """2560-col dense layer as FD<=1024 PSUM groups."""
                for gi, g0 in enumerate(range(0, NODES, GW)):
                    gw = min(GW, NODES - g0)
                    ps = psA.tile([128, GW], f32, tag="psA")
                    for o in range(0, gw, 512):
                        nw = min(512, gw - o)
                        nc.tensor.matmul(ps[:, o:o + nw], w_ap,
                                         src[:, g0 + o:g0 + o + nw],
                                         start=True, stop=True)
                    evict(engines[gi], dst[:, g0:g0 + gw], ps[:, :gw],
                          bias(bias_idx), relu, scale=scale)

            # ---- node encoder + edge layer-1 node halves (w_in2 folded in);
            # u before v: u's eviction tail overlaps v's matmuls, and the
            # adds only need v's first group to start.
            node_layer(W(W1T), x_fm, t_enc, B1, True, EV_ENC)
            pe_filler(1, src=t_enc[:, 0:256])
            node_layer(W(WE1T), t_enc, u_t, BU, False, EV_U)
            pe_filler(1, src=u_t[:, 0:256])
            node_layer(W(WE1B), t_enc, v_t, BV, False, EV_V)
            pe_filler(1, src=v_t[:, 0:256])

            # ---- h1 = relu(u[s] + v[r]) over (r, s', p) columns, where the
            # 9 sender slots s' skip s == r (no diagonal is ever computed).
            v3 = v_t[:].rearrange("f (r p) -> f r p", p=NP_CORE)

            def tt_r(r):
                w0 = r * QB
                vb1 = v3[:, r:r + 1, :]
                ranges = [(0, r, w0), (r + 1, A, w0 + r * NP_CORE)]
                if r < 2:
                    # split the long range at the u-eviction-group boundary
                    # (s-block 8 = u col 2048) so the first piece only waits
                    # on u groups 0-1 and the adds start ~1us earlier
                    lo, hi, d0 = ranges.pop()
                    ranges += [(lo, 8, d0), (8, hi, d0 + (8 - lo) * NP_CORE)]
                first = True
                for lo, hi, d0 in ranges:
                    k = hi - lo
                    if k == 0:
                        continue
                    o = h1_t[:, d0:d0 + k * NP_CORE] \
                        .rearrange("f (s p) -> f s p", p=NP_CORE)
                    us = u_t[:, lo * NP_CORE:hi * NP_CORE] \
                        .rearrange("f (s p) -> f s p", p=NP_CORE)
                    nc.vector.tensor_add(o, us,
                                         vb1.broadcast_to([128, k, NP_CORE]))
                    if r < 2 and first:
                        pe_filler(1, w_ap=W(WE2S),
                                  src=h1_t[:, d0:d0 + 128])
                    first = False

                def relu_part(flat):
                    eng = RELU_ENG[r]
                    if eng == "act":
                        nc.scalar.activation(flat, flat, Af.Relu)
                    else:
                        nc.vector.tensor_scalar_max(flat, flat, 0.0)
                if r < 2:
                    # split so the first e2 groups unblock sooner
                    relu_part(h1_t[:, w0:w0 + GW])
                    relu_part(h1_t[:, w0 + GW:w0 + QB])
                else:
                    relu_part(h1_t[:, w0:w0 + QB])

            pe_filler(1, src=u_t[:, 2304:2560])
            pe_filler(1, src=v_t[:, 2304:2560])
            for r in range(A):
                tt_r(r)
                if r < 3:
                    pe_filler(2, src=h1_t[:, r * QB:r * QB + 256])

            # ---- main stream: h2 = relu(w_e2s^T h1 + b_e2s) stored fp8;
            # after the groups covering r-blocks {2c, 2c+1}, the fused
            # agg+n1 chunk runs (fp8 DoubleRow over sender-slot pairs), and
            # per completed t_n1 slab the rest of the network.
            h2v = h2_t[:].rearrange("f (r s p) -> f s r p", s=A - 1,
                                    p=NP_CORE)

            def agg_chunk(c):
                ps = psB.tile([128, 512], f32, tag="psB")
                nc.tensor.matmul(ps[:], W(WN1AS),
                                 t_enc[:, c * 512:(c + 1) * 512],
                                 start=True, stop=False)
                for ri, r in enumerate((2 * c, 2 * c + 1)):
                    po = ri * NP_CORE
                    for a2 in range(4):
                        nc.tensor.matmul(
                            ps[:, po:po + NP_CORE], wdr_pair,
                            h2v[:, 2 * a2:2 * a2 + 2, r:r + 1, :],
                            start=False, stop=False, perf_mode=DR)
                    nc.tensor.matmul(ps[:, po:po + NP_CORE], wdr_one,
                                     h2v[:, 8:9, r:r + 1, :],
                                     start=False, stop=(ri == 1))
                evict("act", t_n1[:, c * 512:(c + 1) * 512], ps[:],
                      bias(BN1), True, scale=S_N1)

            def slab(s0, sw, step):
                """node-MLP tail + decoder for t_n1 cols [s0, s0+sw).
                Stage-major over the slab so each weight loads once."""
                steps = list(range(s0, s0 + sw, step))
                for si, c0 in enumerate(steps):
                    ps = psB.tile([128, 512], f32, tag="psB")
                    nc.tensor.matmul(ps[:, :step], W(WN2),
                                     t_n1[:, c0:c0 + step],
                                     start=True, stop=True)
                    evict(EV_N2, t_n2[:, c0:c0 + step], ps[:, :step],
                          bias(BN2), True)
                for si, c0 in enumerate(steps):
                    ps = psB.tile([128, 512], f32, tag="psB")
                    nc.tensor.matmul(ps[:, :step], W(WL1),
                                     t_n2[:, c0:c0 + step],
                                     start=True, stop=True)
                    evict(EV_L1[(c0 // 512) % len(EV_L1)],
                          t_l1[:, c0:c0 + step], ps[:, :step],
                          bias(BL1), True)
                for si, c0 in enumerate(steps):
                    ps = psB.tile([128, 512], f32, tag="psB")
                    nc.tensor.matmul(ps[:, :step], W(WL2),
                                     t_l1[:, c0:c0 + step],
                                     start=True, stop=True)
                    if EV_OUT == "act":
                        nc.scalar.activation(out_sb[:, c0:c0 + step],
                                             ps[:ACT, :step], Af.Identity,
                                             bias=bt[0:ACT, BL2:BL2 + 1])
                    else:
                        nc.vector.tensor_scalar_add(out_sb[:, c0:c0 + step],
                                                    ps[:ACT, :step],
                                                    bt[0:ACT, BL2:BL2 + 1])
                    nc.sync.dma_start(out=out_dram[:, c0:c0 + step],
                                      in_=out_sb[:, c0:c0 + step])

            # agg+n1 chunk c needs h2 r-blocks {2c, 2c+1} = cols up to
            # (2c+2)*QB; fire it after the GW-col group covering that.
            agg_after = {((2 * c + 2) * QB - 1) // GW: c for c in range(5)}
            for g in range(N_EG):
                g0 = g * GW
                gw = min(GW, ECOLS - g0)
                ps = psA.tile([128, GW], f32, tag="psA")
                for o in range(0, gw, 512):
                    nw = min(512, gw - o)
                    nc.tensor.matmul(ps[:, o:o + nw], W(WE2S),
                                     h1_t[:, g0 + o:g0 + o + nw],
                                     start=True, stop=True)
                evict(EV_H2[g], h2_t[:, g0:g0 + gw], ps[:, :gw],
                      bias(BE2S), True)
                # zero-LDWEIGHTS filler (stationary stays WE2S), gated on
                # freshly written h1 so it lands here in the PE stream
                if g + 1 < N_EG:
                    pe_filler(1, w_ap=W(WE2S), src=h1_t[:, g0:g0 + 128])
                if g in agg_after:
                    c = agg_after[g]
                    agg_chunk(c)
                    pe_filler(1, w_ap=W(WE2S), src=t_n1[:, c * 512:c * 512 + 128])
                    if c == 1:
                        slab(0, 1024, 512)
                    elif c == 3:
                        slab(1024, 1024, 512)
                    elif c == 4:
                        slab(2048, 512, 256)

    nc.compile()
    _dedupe_ldweights(nc)
    return nc


def _dedupe_ldweights(nc):
    """Remove redundant PE weight loads after bacc splits matmuls into
    Ldweights+Matmult pairs: a Ldweights whose source AP equals the
    previously loaded one (PE stream order == block order) is a no-op.
    Only drop instructions carrying no semaphore waits/updates."""
    from concourse import mybir
    import bass_rust
    for f in nc.m.functions:
        for b in f.blocks:
            last = None
            keep = []
            insts = b.instructions
            for idx, i in enumerate(insts):
                if isinstance(i, mybir.InstLdweights):
                    key = str(i.ins[0])
                    if key == last:
                        if i.sync_info is None:
                            continue
                        # migrate waits/updates onto the paired matmult so
                        # the redundant load can still be dropped
                        nxt = insts[idx + 1] if idx + 1 < len(insts) else None
                        if isinstance(nxt, mybir.InstMatmult):
                            ow = list(i.sync_info.on_wait)
                            ou = list(i.sync_info.on_update)
                            if nxt.sync_info is not None:
                                ow += list(nxt.sync_info.on_wait)
                                ou += list(nxt.sync_info.on_update)
                            if len(ow) <= 1:    # walrus: one wait per inst
                                nxt.sync_info = bass_rust.SyncInfo(
                                    on_wait=ow, on_update=ou)
                                continue
                    last = key
                keep.append(i)
            if len(keep) != len(insts):
                b.instructions[:] = keep


def _get_program():
    global _PROG
    if _PROG is None:
        _PROG = _build_program()
    return _PROG


# ------------------------------------------------------------------- kernel

def kernel(theta, s, i, senders, receivers,
           w_in1, b_in1, w_in2, b_in2,
           w_e1, b_e1, w_e2, b_e2, w_e3, b_e3,
           w_n1, b_n1, w_n2, b_n2, w_n3, b_n3,
           w_l1, b_l1, w_l2, b_l2):
    global LAST_EXEC_NS
    import os
    import ml_dtypes

    args = dict(theta=theta, s=s, i=i, senders=senders, receivers=receivers,
                w_in1=w_in1, b_in1=b_in1, w_in2=w_in2, b_in2=b_in2,
                w_e1=w_e1, b_e1=b_e1, w_e2=w_e2, b_e2=b_e2,
                w_e3=w_e3, b_e3=b_e3, w_n1=w_n1, b_n1=b_n1,
                w_n2=w_n2, b_n2=b_n2, w_n3=w_n3, b_n3=b_n3,
                w_l1=w_l1, b_l1=b_l1, w_l2=w_l2, b_l2=b_l2)
    args = {k: np.asarray(v) for k, v in args.items()}

    # The device program hardcodes the documented block-diagonal
    # fully-connected edge structure and constant-i input; verify, else
    # fall back to a host fp32 computation (correct for any input).
    exp_s, exp_r = _expected_edges()
    i_flat = np.asarray(args["i"], np.float32).reshape(-1)
    structured = (np.array_equal(np.asarray(args["senders"], np.int64), exp_s)
                  and np.array_equal(np.asarray(args["receivers"], np.int64), exp_r)
                  and np.all(i_flat == i_flat[0]))
    if not structured:
        return _fallback_numpy(**{k: np.asarray(v, np.float32)
                                  if np.asarray(v).dtype != np.int32 else np.asarray(v)
                                  for k, v in args.items()})

    f64 = np.float64
    fp8 = ml_dtypes.float8_e4m3
    w_in1_, b_in1_ = args["w_in1"].astype(f64), args["b_in1"].astype(f64)
    w_in2_, b_in2_ = args["w_in2"].astype(f64), args["b_in2"].astype(f64)
    w_e1_, b_e1_ = args["w_e1"].astype(f64), args["b_e1"].astype(f64)
    w_e3_, b_e3_ = args["w_e3"].astype(f64), args["b_e3"].astype(f64)
    w_n1_, b_n1_ = args["w_n1"].astype(f64), args["b_n1"].astype(f64)
    w_n3_, b_n3_ = args["w_n3"].astype(f64), args["b_n3"].astype(f64)
    w_l1_, b_l1_ = args["w_l1"].astype(f64), args["b_l1"].astype(f64)

    b1_eff = b_in1_ + i_flat[0] * w_in1_[H_DIM + S_DIM]
    b_u = b_e1_ + b_in2_ @ w_e1_[:MID]
    b_v = b_in2_ @ w_e1_[MID:]
    b_n1_eff = b_n1_ + b_in2_ @ w_n1_[:MID] + b_e3_ @ w_n1_[MID:]
    b_l1_eff = b_l1_ + b_n3_ @ w_l1_

    wn1b = (w_e3_ / (A - 1)) @ w_n1_[MID:]
    wslots = np.zeros((NSLOTS, 128, 128), np.float16)
    wslots[W1T] = w_in1_[:128].astype(np.float16)
    wslots[WE1T] = (w_in2_ @ w_e1_[:MID]).astype(np.float16)
    wslots[WE1B] = (w_in2_ @ w_e1_[MID:]).astype(np.float16)
    wslots[WE2S] = (args["w_e2"].astype(f64) * S_H2).astype(np.float16)
    wslots[WN1AS] = ((w_in2_ @ w_n1_[:MID]) * (S_H2 * S_WB)).astype(np.float16)
    wslots[WN2] = args["w_n2"].astype(np.float16)
    wslots[WL1] = (w_n3_ @ w_l1_).astype(np.float16)
    wslots[WL2, :, :ACT] = args["w_l2"].astype(np.float16)
    w_pack = np.ascontiguousarray(
        wslots.transpose(1, 0, 2).reshape(128, NSLOTS * 128))

    wn1b8 = np.clip(wn1b * S_WB, -240.0, 240.0).astype(fp8)
    wdr_pack = np.ascontiguousarray(
        np.concatenate([wn1b8, wn1b8, wn1b8], axis=1))          # [128, 384]

    b_pack = np.zeros((128, 8), np.float32)
    for idx, vec in ((B1, b1_eff), (BU, b_u), (BV, b_v),
                     (BE2S, args["b_e2"].astype(f64) * S_H2),
                     (BN1, b_n1_eff), (BN2, args["b_n2"]), (BL1, b_l1_eff)):
        b_pack[:, idx] = np.asarray(vec, np.float32)
    b_pack[:ACT, BL2] = args["b_l2"].astype(np.float32)

    # node features, feat-major, (a, p) column order, per-core shards
    n_all = B * P * A
    X = np.concatenate([args["theta"].reshape(n_all, H_DIM),
                        args["s"].reshape(n_all, S_DIM)], axis=-1)
    in_maps = []
    for c in range(N_CORES):
        xc = X[c * NODES:(c + 1) * NODES]
        in_maps.append({
            "x_fm": _to_ap_major(xc).astype(np.float16),
            "w_pack": w_pack,
            "wdr_pack": wdr_pack,
            "b_pack": b_pack,
        })

    nc = _get_program()
    if os.environ.get("KERNEL_SIM", "0") == "1":
        # CoreSim core 0 only (cores are identical up to data); other cores
        # return zeros. For correctness devloop, not grading.
        from concourse import bass_interp
        sim = bass_interp.CoreSim(nc)
        for k, v in in_maps[0].items():
            sim.tensor(k)[:] = v
        sim.simulate()
        results = [{"out": np.array(sim.tensor("out"))}]
        results += [{"out": np.zeros((ACT, NODES), np.float32)}
                    for _ in range(N_CORES - 1)]
        parts = [_from_ap_major(r["out"]) for r in results]
        return np.concatenate(parts, axis=0).reshape(B, P, A, ACT).astype(np.float32)

    from concourse.bass_utils import run_bass_kernel_spmd
    trace = os.environ.get("KERNEL_TRACE", "0") == "1"
    res = run_bass_kernel_spmd(nc, in_maps, core_ids=list(range(N_CORES)),
                               trace=trace)
    LAST_EXEC_NS = res.exec_time_ns

    parts = [_from_ap_major(res.results[c]["out"]) for c in range(N_CORES)]
    return np.concatenate(parts, axis=0).reshape(B, P, A, ACT).astype(np.float32)
